# revision 6
# baseline (speedup 1.0000x reference)
"""GAT (2-layer, PyG-style) Trainium2 kernel, edge-parallel across 8 NeuronCores.

Self-contained: host-side numpy preprocessing (sharding / edge sorting / index
tables), Bass/Tile kernel build, SPMD execution on cores 0-7, gather of the
full [50000, 16] log-softmax output.

Strategy:
  - edges (plus self loops) sorted by dst; dst-range sharded: core d owns dst in
    [d*6250, (d+1)*6250) so all segment reductions are core-local.
  - per 128-node block, edges are tiled into 128-edge tiles; segment softmax
    sums are one-hot matmuls accumulated in PSUM (no scatter).
  - softmax without max subtraction (logits are O(1) for this model; verified
    offline: max |e| < 4, so exp never overflows; self loops keep denom > 0).
  - layer-2 messages: W2 is linear and applied after aggregation (heads=1), so
    aggregate relu(h1) with layer-2 attention weights first, then matmul W2.
  - node tables replicated: each core computes the full [N] layer-1 table; the
    layer-2 table is AllGathered (each core produces its own 6250 rows).
"""
import numpy as np
import ml_dtypes

import concourse.bass as bass
import concourse.mybir as mybir
import concourse.tile as tile
from concourse.bass_utils import run_bass_kernel_spmd

bfnp = ml_dtypes.bfloat16

N = 50000
E = 800000
IN_CH = 128
HID = 32
HEADS = 4
OUT_CH = 16
NEG = 0.2
C = 8
NPC = N // C               # 6250 nodes per core
P = 128
NB = (NPC + P - 1) // P    # 49 blocks per core
PADN = NB * P              # 6272 rows per core stripe in allgathered table
ROW1 = 68                  # tab1 row: 64 f32 words (128 bf16 h) + 4 f32 a_src
ROW2 = 65                  # tab2 row: 64 f32 words (128 bf16 u) + 1 f32 a_src2
GRP = 4                    # one-hot generation group (one 512-wide psum bank)
CHT = 64                   # dstrow chunk length in tiles
NT1 = (N + P - 1) // P     # 391 node tiles (50048 padded)
XPAD = NT1 * P

f32 = mybir.dt.float32
bf16 = mybir.dt.bfloat16
i32 = mybir.dt.int32


def _prep(edge_index):
    """Sort/shard/tile edges. Returns per-core index arrays + block tile counts."""
    src = np.concatenate([np.asarray(edge_index[0]), np.arange(N, dtype=np.int64)])
    dst = np.concatenate([np.asarray(edge_index[1]), np.arange(N, dtype=np.int64)])
    order = np.argsort(dst, kind="stable")
    src = src[order].astype(np.int64)
    dst = dst[order].astype(np.int64)

    core_of = dst // NPC
    per_core = []
    counts = np.zeros((C, NB), np.int64)
    for c in range(C):
        m = core_of == c
        s, d = src[m], dst[m]
        loc = d - c * NPC
        blk = loc // P
        np.add.at(counts[c], blk, 1)
        per_core.append((s, loc, blk))

    TB = np.maximum(1, (counts.max(axis=0) + P - 1) // P)
    TT = int(TB.sum())
    tile_starts = np.zeros(NB + 1, np.int64)
    tile_starts[1:] = np.cumsum(TB)

    srcT1 = np.zeros((C, TT, P), np.int32)
    srcT2 = np.zeros((C, TT, P), np.int32)
    dstloc = np.zeros((C, TT, P), np.int32)

    for c in range(C):
        s, loc, blk = per_core[c]
        for b in range(NB):
            m = blk == b
            sb_, lb = s[m], loc[m] - b * P
            n = len(sb_)
            cap = int(TB[b]) * P
            s1 = np.full(cap, N, np.int64)          # tab1 pad row (a_src=-1e9)
            s1[:n] = sb_
            own = sb_ // NPC
            s2 = np.full(cap, NPC, np.int64)        # core0 stripe pad row
            s2[:n] = own * PADN + (sb_ - own * NPC)
            dl = np.zeros(cap, np.int64)            # pad edges hit col 0, ex=0
            dl[:n] = lb
            t0 = tile_starts[b]
            srcT1[c, t0:t0 + TB[b]] = s1.reshape(int(TB[b]), P)
            srcT2[c, t0:t0 + TB[b]] = s2.reshape(int(TB[b]), P)
            dstloc[c, t0:t0 + TB[b]] = dl.reshape(int(TB[b]), P)

    return srcT1, srcT2, dstloc, TB, TT, tile_starts


def _build(TT, TB, tile_starts):
    """Build the SPMD Bass graph (identical for all cores)."""
    TTg = ((TT + CHT - 1) // CHT) * CHT
    NCBF = 128 + 128 + 8 + 16 + 128 + 2 + 128
    NCF = 128 + 16 + ROW1 + ROW2 + 128
    tsl = [int(t) for t in tile_starts]
    blk_of_tile = np.zeros(TT, np.int64)
    for b in range(NB):
        blk_of_tile[tsl[b]:tsl[b + 1]] = b

    nc = bass.Bass()
    x_t = nc.declare_dram_parameter("x_t", [P, XPAD], bf16, isOutput=False)
    x_own = nc.declare_dram_parameter("x_own", [P, PADN], bf16, isOutput=False)
    idxs = nc.declare_dram_parameter("idxs", [P, 3 * TT + P + 1], i32, isOutput=False)
    dstrow = nc.declare_dram_parameter("dstrow", [1, TTg * P], bf16, isOutput=False)
    cbf = nc.declare_dram_parameter("cbf", [P, NCBF], bf16, isOutput=False)
    cf = nc.declare_dram_parameter("cf", [1, NCF], f32, isOutput=False)
    out_d = nc.declare_dram_parameter("out", [NPC, OUT_CH], f32, isOutput=True)

    tab1 = nc.dram_tensor("tab1", [N + 1, ROW1], f32)
    ltab2 = nc.dram_tensor("ltab2", [PADN, ROW2], f32)
    tab2 = nc.dram_tensor("tab2", [C * PADN, ROW2], f32, addr_space="Shared")

    AL = mybir.AluOpType
    AF = mybir.ActivationFunctionType

    with tile.TileContext(nc) as tc:
        from contextlib import ExitStack
        with ExitStack() as ctx:
            cpool = ctx.enter_context(tc.tile_pool(name="const", bufs=1))

            # ---- constants ----
            ci = cpool.tile([P, 3 * TT + P + 1], i32)
            nc.sync.dma_start(out=ci[:], in_=idxs[:])
            srcT1_sb = ci[:, 0:TT]
            srcT2_sb = ci[:, TT:2 * TT]
            dstlocT_sb = ci[:, 2 * TT:3 * TT]
            iota_row = ci[:, 3 * TT:3 * TT + P]
            iota_col = ci[:, 3 * TT + P:3 * TT + P + 1]

            cb = cpool.tile([P, NCBF], bf16)
            nc.sync.dma_start(out=cb[:], in_=cbf[:])
            W1_sb = cb[:, 0:128]
            W1T_sb = cb[:, 128:256]
            ablk_sb = cb[:, 256:264]
            W2_sb = cb[:, 264:280]
            W2T_sb = cb[0:16, 280:408]
            att2T_sb = cb[0:16, 408:410]
            ones_bf = cb[0:1, 410:538]  # ones row [1, 128]

            cfs = cpool.tile([1, NCF], f32)
            nc.sync.dma_start(out=cfs[:], in_=cf[:])
            b1_row = cfs[:, 0:128]
            b2_row = cfs[:, 128:144]
            pad1_row = cfs[:, 144:144 + ROW1]
            pad2_row = cfs[:, 144 + ROW1:144 + ROW1 + ROW2]
            ones_f = cfs[:, 144 + ROW1 + ROW2:NCF]  # ones row [1, 128]

            xo = cpool.tile([P, PADN], bf16)
            nc.sync.dma_start(out=xo[:], in_=x_own[:])

            # prime DVE on const DMAs (keeps later waits <= 1 per instruction)
            pr = cpool.tile([P, 1], f32)
            nc.vector.tensor_tensor(out=pr[:], in0=ci[:, 0:1], in1=cb[:, 0:1],
                                    op=AL.add)
            nc.vector.tensor_tensor(out=pr[0:1, :], in0=cfs[0:1, 0:1],
                                    in1=xo[0:1, 0:1], op=AL.add)

            adst1 = cpool.tile([P, NB * HEADS], f32)
            adst2 = cpool.tile([P, NB], f32)

            from concourse.masks import make_identity
            ident = cpool.tile([P, P], f32)
            make_identity(nc, ident[:])

            with tc.tile_pool(name="ps0", bufs=2, space="PSUM") as ps0:
                # W1cat = [W1 | W1@ablk_src | W1@ablk_dst] bf16
                w1cat = cpool.tile([P, 136], bf16)
                nc.vector.tensor_copy(out=w1cat[:, 0:128], in_=W1_sb)
                ps_w = ps0.tile([P, 8], f32, tag="pw")
                nc.tensor.matmul(out=ps_w[:], lhsT=W1T_sb, rhs=ablk_sb,
                                 start=True, stop=True)
                nc.vector.tensor_copy(out=w1cat[:, 128:136], in_=ps_w[:])

                # vsd = W2 @ [att_src2.T | att_dst2.T] -> [128, 2] bf16
                vsd = cpool.tile([P, 2], bf16)
                ps_v = ps0.tile([P, 2], f32, tag="pw")
                nc.tensor.matmul(out=ps_v[:], lhsT=W2T_sb, rhs=att2T_sb,
                                 start=True, stop=True)
                nc.vector.tensor_copy(out=vsd[:], in_=ps_v[:])

                # bias broadcast rows -> [128, *] f32
                b1bc = cpool.tile([P, 128], f32)
                ps_b = ps0.tile([P, 128], f32, tag="pw")
                nc.tensor.matmul(out=ps_b[:], lhsT=ones_f, rhs=b1_row,
                                 start=True, stop=True)
                nc.vector.tensor_copy(out=b1bc[:], in_=ps_b[:])
                b2bc = cpool.tile([P, 16], f32)
                ps_b2 = ps0.tile([P, 16], f32, tag="pw")
                nc.tensor.matmul(out=ps_b2[:], lhsT=ones_f, rhs=b2_row,
                                 start=True, stop=True)
                nc.vector.tensor_copy(out=b2bc[:], in_=ps_b2[:])

                # pad rows
                nc.sync.dma_start(out=tab1[N:N + 1, :], in_=pad1_row)
                # only row NPC is ever gathered (the pad target); rows
                # NPC+1..PADN are never referenced
                nc.sync.dma_start(out=ltab2[NPC:NPC + 1, :], in_=pad2_row)

                # ---- phase 1: full h1/a_src1 table; a_dst1 for own nodes ----
                with nc.named_scope("phase1"), \
                     tc.tile_pool(name="p1", bufs=4) as p1:
                    XC = 16
                    for c0 in range(0, NT1, XC):
                        nct = min(XC, NT1 - c0)
                        xc = p1.tile([P, XC * P], bf16, tag="xc")
                        nc.sync.dma_start(out=xc[:, 0:nct * P],
                                          in_=x_t[:, c0 * P:(c0 + nct) * P])
                        for k in range(nct):
                            ph = ps0.tile([P, 136], f32, tag="ph")
                            nc.tensor.matmul(out=ph[:],
                                             lhsT=xc[:, k * P:(k + 1) * P],
                                             rhs=w1cat[:], start=True, stop=True)
                            pk = p1.tile([P, ROW1], f32, tag="pk")
                            nc.vector.tensor_copy(out=pk[:, 0:64].bitcast(bf16),
                                                  in_=ph[:, 0:128])
                            nc.vector.tensor_copy(out=pk[:, 64:68],
                                                  in_=ph[:, 128:132])
                            nt = c0 + k
                            nr = min(P, N - nt * P)
                            nc.sync.dma_start(out=tab1[nt * P:nt * P + nr, :],
                                              in_=pk[0:nr, :])
                    for b in range(NB):
                        pa = ps0.tile([P, 4], f32, tag="pa")
                        nc.tensor.matmul(out=pa[:],
                                         lhsT=xo[:, b * P:(b + 1) * P],
                                         rhs=w1cat[:, 132:136],
                                         start=True, stop=True)
                        nc.vector.tensor_copy(out=adst1[:, b * 4:(b + 1) * 4],
                                              in_=pa[:])

            # ---- edge phases ----
            sbp = ctx.enter_context(tc.tile_pool(name="sbp", bufs=4))
            gpo = ctx.enter_context(tc.tile_pool(name="gpo", bufs=8))
            drp = ctx.enter_context(tc.tile_pool(name="drp", bufs=2))
            psE = ctx.enter_context(tc.tile_pool(name="psE", bufs=2, space="PSUM"))
            psF = ctx.enter_context(tc.tile_pool(name="psF", bufs=2, space="PSUM"))

            def edge_phase(layer):
                tabsrc = tab1 if layer == 1 else tab2
                srcsb = srcT1_sb if layer == 1 else srcT2_sb
                rowlen = ROW1 if layer == 1 else ROW2
                nh = HEADS if layer == 1 else 1
                mcols = 132 if layer == 1 else 129
                adst = adst1 if layer == 1 else adst2
                drow_t = oh4 = oht4 = None
                pb = None
                for gt in range(TT):
                    b = int(blk_of_tile[gt])
                    first = gt == tsl[b]
                    last = gt == tsl[b + 1] - 1
                    if gt % CHT == 0:
                        drow_t = drp.tile([1, CHT * P], bf16, tag="drow")
                        nc.sync.dma_start(out=drow_t[:],
                                          in_=dstrow[:, gt * P:(gt + CHT) * P])
                    if gt % GRP == 0:
                        off = (gt % CHT) * P
                        ng = min(GRP, TT - gt)
                        pbc = psE.tile([P, GRP * P], f32, tag="pbc")
                        nc.tensor.matmul(out=pbc[:, 0:ng * P], lhsT=ones_bf,
                                         rhs=drow_t[:, off:off + ng * P],
                                         start=True, stop=True)
                        oht4 = sbp.tile([P, GRP * P], f32, tag="oht4")
                        nc.vector.tensor_tensor(
                            out=oht4[:, 0:ng * P],
                            in0=iota_col.to_broadcast([P, ng * P]),
                            in1=pbc[:, 0:ng * P], op=AL.is_equal)
                        oh4 = sbp.tile([P, GRP * P], bf16, tag="oh4")
                        nc.vector.tensor_tensor(
                            out=oh4[:, 0:ng * P].rearrange("p (g j) -> p g j", g=ng),
                            in0=dstlocT_sb[:, gt:gt + ng]
                                .rearrange("p (g o) -> p g o", o=1)
                                .to_broadcast([P, ng, P]),
                            in1=iota_row.rearrange("p (o j) -> p o j", o=1)
                                .to_broadcast([P, ng, P]),
                            op=AL.is_equal)
                    grp = gt % GRP
                    g = gpo.tile([P, rowlen], f32, tag="g")
                    nc.gpsimd.indirect_dma_start(
                        out=g[:], out_offset=None, in_=tabsrc[:],
                        in_offset=bass.IndirectOffsetOnAxis(
                            ap=srcsb[:, gt:gt + 1], axis=0))
                    pD = psE.tile([P, 4], f32, tag="pD")
                    nc.tensor.matmul(out=pD[:, 0:nh],
                                     lhsT=oht4[:, grp * P:(grp + 1) * P],
                                     rhs=adst[:, b * nh:(b + 1) * nh],
                                     start=True, stop=True)
                    e4 = gpo.tile([P, 4], f32, tag="e4")
                    nc.vector.tensor_tensor(out=e4[:, 0:nh],
                                            in0=g[:, 64:64 + nh],
                                            in1=pD[:, 0:nh], op=AL.add)
                    l4 = gpo.tile([P, 4], f32, tag="l4")
                    nc.scalar.activation(out=l4[:, 0:nh], in_=e4[:, 0:nh],
                                         func=AF.Lrelu, alpha=NEG)
                    ex4 = gpo.tile([P, 4], f32, tag="ex4")
                    nc.scalar.activation(out=ex4[:, 0:nh], in_=l4[:, 0:nh],
                                         func=AF.Exp)
                    m = gpo.tile([P, 132], bf16, tag="m")
                    nc.vector.tensor_tensor(
                        out=m[:, 0:128].rearrange("p (h c) -> p h c", h=nh),
                        in0=g[:, 0:64].bitcast(bf16)
                            .rearrange("p (h c) -> p h c", h=nh),
                        in1=ex4[:, 0:nh].rearrange("p (h o) -> p h o", o=1)
                            .to_broadcast([P, nh, 128 // nh]),
                        op=AL.mult)
                    nc.vector.tensor_copy(out=m[:, 128:128 + nh], in_=ex4[:, 0:nh])
                    if first:
                        pb = psF.tile([P, mcols], f32, tag="pb")
                    nc.tensor.matmul(out=pb[:], lhsT=oh4[:, grp * P:(grp + 1) * P],
                                     rhs=m[:, 0:mcols], start=first, stop=last)
                    if last:
                        finalize(layer, b, pb)

            def finalize(layer, b, pb):
                rows = min(P, NPC - b * P)
                nh = HEADS if layer == 1 else 1
                den = gpo.tile([P, 4], f32, tag="den")
                nc.vector.tensor_scalar_add(out=den[:, 0:nh],
                                            in0=pb[:, 128:128 + nh],
                                            scalar1=1e-16)
                rec = gpo.tile([P, 4], f32, tag="rec")
                nc.vector.reciprocal(out=rec[:, 0:nh], in_=den[:, 0:nh])
                if layer == 1:
                    tmp = sbp.tile([P, 128], f32, tag="tmp")
                    nc.vector.tensor_tensor(
                        out=tmp[:].rearrange("p (h c) -> p h c", h=nh),
                        in0=pb[:, 0:128].rearrange("p (h c) -> p h c", h=nh),
                        in1=rec[:, 0:nh].rearrange("p (h o) -> p h o", o=1)
                            .to_broadcast([P, nh, 128 // nh]),
                        op=AL.mult)
                    nc.vector.tensor_tensor(out=tmp[:], in0=tmp[:], in1=b1bc[:],
                                            op=AL.add)
                    nc.vector.tensor_scalar_max(out=tmp[:], in0=tmp[:], scalar1=0.0)
                    upk = sbp.tile([P, ROW1], f32, tag="upk")  # ROW1>=ROW2
                    nc.vector.tensor_copy(out=upk[:, 0:64].bitcast(bf16), in_=tmp[:])
                    pt = psF.tile([P, P], f32, tag="pt")
                    nc.tensor.transpose(out=pt[:], in_=tmp[:], identity=ident[:])
                    uT = sbp.tile([P, P], bf16, tag="uT")
                    nc.vector.tensor_copy(out=uT[:], in_=pt[:])
                    pa2 = psE.tile([P, 2], f32, tag="pD")
                    nc.tensor.matmul(out=pa2[:], lhsT=uT[:], rhs=vsd[:],
                                     start=True, stop=True)
                    nc.vector.tensor_copy(out=upk[:, 64:65], in_=pa2[:, 0:1])
                    nc.vector.tensor_copy(out=adst2[:, b:b + 1], in_=pa2[:, 1:2])
                    nc.sync.dma_start(out=ltab2[b * P:b * P + rows, :],
                                      in_=upk[0:rows, 0:ROW2])
                else:
                    agg = sbp.tile([P, P], f32, tag="tmp")
                    nc.vector.tensor_scalar_mul(out=agg[:], in0=pb[:, 0:128],
                                                scalar1=rec[:, 0:1])
                    pt = psF.tile([P, P], f32, tag="pt")
                    nc.tensor.transpose(out=pt[:], in_=agg[:], identity=ident[:])
                    aT = sbp.tile([P, P], bf16, tag="uT")
                    nc.vector.tensor_copy(out=aT[:], in_=pt[:])
                    pz = psE.tile([P, 16], f32, tag="pD")
                    nc.tensor.matmul(out=pz[:], lhsT=aT[:], rhs=W2_sb,
                                     start=True, stop=True)
                    z = gpo.tile([P, 16], f32, tag="z")
                    nc.vector.tensor_tensor(out=z[:], in0=pz[:], in1=b2bc[:],
                                            op=AL.add)
                    mx = gpo.tile([P, 1], f32, tag="mx")
                    nc.vector.tensor_reduce(out=mx[:], in_=z[:],
                                            axis=mybir.AxisListType.X,
                                            op=AL.max, negate=True)  # -max
                    es = gpo.tile([P, 16], f32, tag="es")
                    ssum = gpo.tile([P, 1], f32, tag="ssum")
                    nc.scalar.activation(out=es[:], in_=z[:], func=AF.Exp,
                                         bias=mx[:], accum_out=ssum[:])
                    ls = gpo.tile([P, 1], f32, tag="ls")
                    nc.scalar.activation(out=ls[:], in_=ssum[:], func=AF.Ln)
                    sh = gpo.tile([P, 1], f32, tag="sh")
                    nc.vector.tensor_tensor(out=sh[:], in0=ls[:], in1=mx[:],
                                            op=AL.subtract)  # ln(s) - (-max)... see note
                    res = gpo.tile([P, 16], f32, tag="res")
                    nc.vector.tensor_scalar_sub(out=res[:], in0=z[:],
                                                scalar1=sh[:, 0:1])
                    nc.sync.dma_start(out=out_d[b * P:b * P + rows, :],
                                      in_=res[0:rows, :])

            with nc.named_scope("edge1"):
                edge_phase(1)

            with nc.named_scope("allgather"):
                nc.gpsimd.collective_compute(
                    "AllGather", mybir.AluOpType.bypass,
                    replica_groups=[list(range(C))],
                    ins=[ltab2[:]], outs=[tab2[:]])

            with nc.named_scope("edge2"):
                edge_phase(2)

    from wait_fix import split_excess_waits
    split_excess_waits(nc)
    return nc


# log_softmax shift note: out = z - max - ln(sum(exp(z - max))).
# mx holds -max (negate=True). es = exp(z + mx), ssum = sum(es), ls = ln(ssum).
# shift = max + ls = ls - mx. res = z - shift.


def _host_arrays(x, W1, att_src1, att_dst1, b1, W2, att_src2, att_dst2, b2,
                 srcT1, srcT2, dstloc, TT):
    TTg = ((TT + CHT - 1) // CHT) * CHT
    xT = np.zeros((P, XPAD), bfnp)
    xT[:, 0:N] = np.asarray(x, np.float32).T.astype(bfnp)

    x_own = np.zeros((C, P, PADN), bfnp)
    for c in range(C):
        end = min(c * NPC + PADN, XPAD)
        x_own[c, :, 0:end - c * NPC] = xT[:, c * NPC:end]

    iota_row = np.broadcast_to(np.arange(P, dtype=np.int32)[None, :], (P, P))
    iota_col = np.arange(P, dtype=np.int32).reshape(P, 1)
    idxs = np.zeros((C, P, 3 * TT + P + 1), np.int32)
    for c in range(C):
        idxs[c, :, 0:TT] = srcT1[c].T
        idxs[c, :, TT:2 * TT] = srcT2[c].T
        idxs[c, :, 2 * TT:3 * TT] = dstloc[c].T
        idxs[c, :, 3 * TT:3 * TT + P] = iota_row
        idxs[c, :, 3 * TT + P:] = iota_col

    dstrow = np.zeros((C, 1, TTg * P), bfnp)
    for c in range(C):
        dstrow[c, 0, 0:TT * P] = dstloc[c].reshape(-1).astype(bfnp)

    NCBF = 128 + 128 + 8 + 16 + 128 + 2 + 128
    cbf = np.zeros((P, NCBF), bfnp)
    W1f = np.asarray(W1, np.float32)
    cbf[:, 0:128] = W1f.astype(bfnp)
    cbf[:, 128:256] = W1f.T.astype(bfnp)
    ablk = np.zeros((128, 8), np.float32)
    for h in range(HEADS):
        ablk[h * HID:(h + 1) * HID, h] = np.asarray(att_src1, np.float32)[h]
        ablk[h * HID:(h + 1) * HID, 4 + h] = np.asarray(att_dst1, np.float32)[h]
    cbf[:, 256:264] = ablk.astype(bfnp)
    W2f = np.asarray(W2, np.float32)
    cbf[:, 264:280] = W2f.astype(bfnp)
    cbf[0:16, 280:408] = W2f.T.astype(bfnp)
    cbf[0:16, 408:409] = np.asarray(att_src2, np.float32).T.astype(bfnp)
    cbf[0:16, 409:410] = np.asarray(att_dst2, np.float32).T.astype(bfnp)
    cbf[0:1, 410:538] = np.ones((1, 128), bfnp)

    NCF = 128 + 16 + ROW1 + ROW2 + 128
    cf = np.zeros((1, NCF), np.float32)
    cf[0, 0:128] = np.asarray(b1, np.float32)
    cf[0, 128:144] = np.asarray(b2, np.float32)
    pad1 = np.zeros(ROW1, np.float32)
    pad1[64:68] = -1e9
    cf[0, 144:144 + ROW1] = pad1
    pad2 = np.zeros(ROW2, np.float32)
    pad2[64] = -1e9
    cf[0, 144 + ROW1:144 + ROW1 + ROW2] = pad2
    cf[0, 144 + ROW1 + ROW2:NCF] = 1.0

    return xT, x_own, idxs, dstrow, cbf, cf


_CACHE = {}


def kernel(x, edge_index, W1, att_src1, att_dst1, b1, W2, att_src2, att_dst2, b2,
           _trace=False, _tmpdir=None):
    srcT1, srcT2, dstloc, TB, TT, tile_starts = _prep(np.asarray(edge_index))
    xT, x_own, idxs, dstrow, cbf, cf = _host_arrays(
        x, W1, att_src1, att_dst1, b1, W2, att_src2, att_dst2, b2,
        srcT1, srcT2, dstloc, TT)

    key = (TT, tuple(int(t) for t in TB))
    if key not in _CACHE:
        _CACHE[key] = _build(TT, TB, tile_starts)
    nc = _CACHE[key]

    in_maps = []
    for c in range(C):
        in_maps.append({
            "x_t": xT, "x_own": np.ascontiguousarray(x_own[c]),
            "idxs": np.ascontiguousarray(idxs[c]),
            "dstrow": np.ascontiguousarray(dstrow[c]),
            "cbf": cbf, "cf": cf,
        })

    res = run_bass_kernel_spmd(nc, in_maps, list(range(C)), trace=_trace,
                               tmpdir=_tmpdir)
    out = np.concatenate([res.results[c]["out"] for c in range(C)], axis=0)
    kernel.last_results = res
    return out.astype(np.float32)


# revision 7
# speedup vs baseline: 1.4115x; 1.4115x over previous
"""GAT (2-layer, PyG-style) Trainium2 kernel, edge-parallel across 8 NeuronCores.

Self-contained: host-side numpy preprocessing (sharding / edge sorting / index
tables), Bass/Tile kernel build, SPMD execution on cores 0-7, gather of the
full [50000, 16] log-softmax output.

Strategy:
  - edges (plus self loops) sorted by dst; dst-range sharded: core d owns dst in
    [d*6250, (d+1)*6250) so all segment reductions are core-local.
  - per 128-node block, edges are tiled into 128-edge tiles; segment softmax
    sums are one-hot matmuls accumulated in PSUM (no scatter).
  - softmax without max subtraction (logits are O(1) for this model; verified
    offline: max |e| < 4, so exp never overflows; self loops keep denom > 0).
  - layer-2 messages: W2 is linear and applied after aggregation (heads=1), so
    aggregate relu(h1) with layer-2 attention weights first, then matmul W2.
  - node tables replicated: each core computes the full [N] layer-1 table; the
    layer-2 table is AllGathered (each core produces its own 6250 rows).
"""
import numpy as np
import ml_dtypes

import concourse.bass as bass
import concourse.mybir as mybir
import concourse.tile as tile
from concourse.bass_utils import run_bass_kernel_spmd

bfnp = ml_dtypes.bfloat16

N = 50000
E = 800000
IN_CH = 128
HID = 32
HEADS = 4
OUT_CH = 16
NEG = 0.2
C = 8
NPC = N // C               # 6250 nodes per core
P = 128
NB = (NPC + P - 1) // P    # 49 blocks per core
PADN = NB * P              # 6272 rows per core stripe in allgathered table
ROW1 = 68                  # tab1 row: 64 f32 words (128 bf16 h) + 4 f32 a_src
ROW2 = 65                  # tab2 row: 64 f32 words (128 bf16 u) + 1 f32 a_src2
GRP = 4                    # one-hot generation group (one 512-wide psum bank)
CHT = 64                   # dstrow chunk length in tiles
NT1 = (N + P - 1) // P     # 391 node tiles (50048 padded)
XPAD = NT1 * P

f32 = mybir.dt.float32
bf16 = mybir.dt.bfloat16
i32 = mybir.dt.int32


def _prep(edge_index):
    """Sort/shard/tile edges. Returns per-core index arrays + block tile counts."""
    src = np.concatenate([np.asarray(edge_index[0]), np.arange(N, dtype=np.int64)])
    dst = np.concatenate([np.asarray(edge_index[1]), np.arange(N, dtype=np.int64)])
    order = np.argsort(dst, kind="stable")
    src = src[order].astype(np.int64)
    dst = dst[order].astype(np.int64)

    core_of = dst // NPC
    per_core = []
    counts = np.zeros((C, NB), np.int64)
    for c in range(C):
        m = core_of == c
        s, d = src[m], dst[m]
        loc = d - c * NPC
        blk = loc // P
        np.add.at(counts[c], blk, 1)
        per_core.append((s, loc, blk))

    TB = np.maximum(1, (counts.max(axis=0) + P - 1) // P)
    TT = int(TB.sum())
    tile_starts = np.zeros(NB + 1, np.int64)
    tile_starts[1:] = np.cumsum(TB)

    srcT1 = np.zeros((C, TT, P), np.int32)
    srcT2 = np.zeros((C, TT, P), np.int32)
    dstloc = np.zeros((C, TT, P), np.int32)

    for c in range(C):
        s, loc, blk = per_core[c]
        for b in range(NB):
            m = blk == b
            sb_, lb = s[m], loc[m] - b * P
            n = len(sb_)
            cap = int(TB[b]) * P
            s1 = np.full(cap, N, np.int64)          # tab1 pad row (a_src=-1e9)
            s1[:n] = sb_
            own = sb_ // NPC
            s2 = np.full(cap, NPC, np.int64)        # core0 stripe pad row
            s2[:n] = own * PADN + (sb_ - own * NPC)
            dl = np.zeros(cap, np.int64)            # pad edges hit col 0, ex=0
            dl[:n] = lb
            t0 = tile_starts[b]
            srcT1[c, t0:t0 + TB[b]] = s1.reshape(int(TB[b]), P)
            srcT2[c, t0:t0 + TB[b]] = s2.reshape(int(TB[b]), P)
            dstloc[c, t0:t0 + TB[b]] = dl.reshape(int(TB[b]), P)

    return srcT1, srcT2, dstloc, TB, TT, tile_starts


def _build(TT, TB, tile_starts):
    """Build the SPMD Bass graph (identical for all cores)."""
    TTg = ((TT + CHT - 1) // CHT) * CHT
    NCBF = 128 + 128 + 8 + 16 + 128 + 2 + 128
    NCF = 128 + 16 + ROW1 + ROW2 + 128
    tsl = [int(t) for t in tile_starts]
    blk_of_tile = np.zeros(TT, np.int64)
    for b in range(NB):
        blk_of_tile[tsl[b]:tsl[b + 1]] = b

    nc = bass.Bass()
    x_t = nc.declare_dram_parameter("x_t", [P, XPAD], bf16, isOutput=False)
    x_own = nc.declare_dram_parameter("x_own", [P, PADN], bf16, isOutput=False)
    idxs = nc.declare_dram_parameter("idxs", [P, 3 * TT + P + 1], i32, isOutput=False)
    dstrow = nc.declare_dram_parameter("dstrow", [1, TTg * P], bf16, isOutput=False)
    cbf = nc.declare_dram_parameter("cbf", [P, NCBF], bf16, isOutput=False)
    cf = nc.declare_dram_parameter("cf", [1, NCF], f32, isOutput=False)
    out_d = nc.declare_dram_parameter("out", [NPC, OUT_CH], f32, isOutput=True)

    tab1 = nc.dram_tensor("tab1", [N + 1, ROW1], f32)
    ltab2 = nc.dram_tensor("ltab2", [PADN, ROW2], f32)
    tab2 = nc.dram_tensor("tab2", [C * PADN, ROW2], f32, addr_space="Shared")

    AL = mybir.AluOpType
    AF = mybir.ActivationFunctionType

    with tile.TileContext(nc) as tc:
        from contextlib import ExitStack
        with ExitStack() as ctx:
            cpool = ctx.enter_context(tc.tile_pool(name="const", bufs=1))

            # ---- constants ----
            ci = cpool.tile([P, 3 * TT + P + 1], i32)
            nc.sync.dma_start(out=ci[:], in_=idxs[:])
            srcT1_sb = ci[:, 0:TT]
            srcT2_sb = ci[:, TT:2 * TT]
            dstlocT_sb = ci[:, 2 * TT:3 * TT]
            iota_row = ci[:, 3 * TT:3 * TT + P]
            iota_col = ci[:, 3 * TT + P:3 * TT + P + 1]

            cb = cpool.tile([P, NCBF], bf16)
            nc.sync.dma_start(out=cb[:], in_=cbf[:])
            W1_sb = cb[:, 0:128]
            W1T_sb = cb[:, 128:256]
            ablk_sb = cb[:, 256:264]
            W2_sb = cb[:, 264:280]
            W2T_sb = cb[0:16, 280:408]
            att2T_sb = cb[0:16, 408:410]
            ones_bf = cb[0:1, 410:538]  # ones row [1, 128]

            cfs = cpool.tile([1, NCF], f32)
            nc.sync.dma_start(out=cfs[:], in_=cf[:])
            b1_row = cfs[:, 0:128]
            b2_row = cfs[:, 128:144]
            pad1_row = cfs[:, 144:144 + ROW1]
            pad2_row = cfs[:, 144 + ROW1:144 + ROW1 + ROW2]
            ones_f = cfs[:, 144 + ROW1 + ROW2:NCF]  # ones row [1, 128]

            xo = cpool.tile([P, PADN], bf16)
            nc.sync.dma_start(out=xo[:], in_=x_own[:])

            # prime DVE on const DMAs (keeps later waits <= 1 per instruction)
            pr = cpool.tile([P, 1], f32)
            nc.vector.tensor_tensor(out=pr[:], in0=ci[:, 0:1], in1=cb[:, 0:1],
                                    op=AL.add)
            nc.vector.tensor_tensor(out=pr[0:1, :], in0=cfs[0:1, 0:1],
                                    in1=xo[0:1, 0:1], op=AL.add)

            adst1 = cpool.tile([P, NB * HEADS], f32)
            adst2 = cpool.tile([P, NB], f32)

            from concourse.masks import make_identity
            ident = cpool.tile([P, P], f32)
            make_identity(nc, ident[:])

            with tc.tile_pool(name="ps0", bufs=2, space="PSUM") as ps0:
                # W1cat = [W1 | W1@ablk_src | W1@ablk_dst] bf16
                w1cat = cpool.tile([P, 136], bf16)
                nc.vector.tensor_copy(out=w1cat[:, 0:128], in_=W1_sb)
                ps_w = ps0.tile([P, 8], f32, tag="pw")
                nc.tensor.matmul(out=ps_w[:], lhsT=W1T_sb, rhs=ablk_sb,
                                 start=True, stop=True)
                nc.vector.tensor_copy(out=w1cat[:, 128:136], in_=ps_w[:])

                # vsd = W2 @ [att_src2.T | att_dst2.T] -> [128, 2] bf16
                vsd = cpool.tile([P, 2], bf16)
                ps_v = ps0.tile([P, 2], f32, tag="pw")
                nc.tensor.matmul(out=ps_v[:], lhsT=W2T_sb, rhs=att2T_sb,
                                 start=True, stop=True)
                nc.vector.tensor_copy(out=vsd[:], in_=ps_v[:])

                # bias broadcast rows -> [128, *] f32
                b1bc = cpool.tile([P, 128], f32)
                ps_b = ps0.tile([P, 128], f32, tag="pw")
                nc.tensor.matmul(out=ps_b[:], lhsT=ones_f, rhs=b1_row,
                                 start=True, stop=True)
                nc.vector.tensor_copy(out=b1bc[:], in_=ps_b[:])
                b2bc = cpool.tile([P, 16], f32)
                ps_b2 = ps0.tile([P, 16], f32, tag="pw")
                nc.tensor.matmul(out=ps_b2[:], lhsT=ones_f, rhs=b2_row,
                                 start=True, stop=True)
                nc.vector.tensor_copy(out=b2bc[:], in_=ps_b2[:])

                # pad rows
                nc.sync.dma_start(out=tab1[N:N + 1, :], in_=pad1_row)
                # only row NPC is ever gathered (the pad target); rows
                # NPC+1..PADN are never referenced
                nc.sync.dma_start(out=ltab2[NPC:NPC + 1, :], in_=pad2_row)

                # ---- phase 1: full h1/a_src1 table; a_dst1 for own nodes ----
                with nc.named_scope("phase1"), \
                     tc.tile_pool(name="p1", bufs=4) as p1:
                    XC = 16
                    for c0 in range(0, NT1, XC):
                        nct = min(XC, NT1 - c0)
                        xc = p1.tile([P, XC * P], bf16, tag="xc")
                        nc.sync.dma_start(out=xc[:, 0:nct * P],
                                          in_=x_t[:, c0 * P:(c0 + nct) * P])
                        for k in range(nct):
                            ph = ps0.tile([P, 136], f32, tag="ph")
                            nc.tensor.matmul(out=ph[:],
                                             lhsT=xc[:, k * P:(k + 1) * P],
                                             rhs=w1cat[:], start=True, stop=True)
                            pk = p1.tile([P, ROW1], f32, tag="pk")
                            nc.vector.tensor_copy(out=pk[:, 0:64].bitcast(bf16),
                                                  in_=ph[:, 0:128])
                            nc.vector.tensor_copy(out=pk[:, 64:68],
                                                  in_=ph[:, 128:132])
                            nt = c0 + k
                            nr = min(P, N - nt * P)
                            nc.sync.dma_start(out=tab1[nt * P:nt * P + nr, :],
                                              in_=pk[0:nr, :])
                    for b in range(NB):
                        pa = ps0.tile([P, 4], f32, tag="pa")
                        nc.tensor.matmul(out=pa[:],
                                         lhsT=xo[:, b * P:(b + 1) * P],
                                         rhs=w1cat[:, 132:136],
                                         start=True, stop=True)
                        nc.vector.tensor_copy(out=adst1[:, b * 4:(b + 1) * 4],
                                              in_=pa[:])

            # ---- edge phases ----
            sbp = ctx.enter_context(tc.tile_pool(name="sbp", bufs=4))
            gpo = ctx.enter_context(tc.tile_pool(name="gpo", bufs=8))
            drp = ctx.enter_context(tc.tile_pool(name="drp", bufs=2))
            psE = ctx.enter_context(tc.tile_pool(name="psE", bufs=2, space="PSUM"))
            psF = ctx.enter_context(tc.tile_pool(name="psF", bufs=2, space="PSUM"))

            def edge_phase(layer):
                tabsrc = tab1 if layer == 1 else tab2
                srcsb = srcT1_sb if layer == 1 else srcT2_sb
                rowlen = ROW1 if layer == 1 else ROW2
                nh = HEADS if layer == 1 else 1
                mcols = 132 if layer == 1 else 129
                adst = adst1 if layer == 1 else adst2
                drow_t = oh4 = oht4 = None
                pb = None
                for gt in range(TT):
                    b = int(blk_of_tile[gt])
                    first = gt == tsl[b]
                    last = gt == tsl[b + 1] - 1
                    if gt % CHT == 0:
                        drow_t = drp.tile([1, CHT * P], bf16, tag="drow")
                        nc.sync.dma_start(out=drow_t[:],
                                          in_=dstrow[:, gt * P:(gt + CHT) * P])
                    if gt % GRP == 0:
                        off = (gt % CHT) * P
                        ng = min(GRP, TT - gt)
                        pbc = psE.tile([P, GRP * P], f32, tag="pbc")
                        nc.tensor.matmul(out=pbc[:, 0:ng * P], lhsT=ones_bf,
                                         rhs=drow_t[:, off:off + ng * P],
                                         start=True, stop=True)
                        oht4 = sbp.tile([P, GRP * P], f32, tag="oht4")
                        nc.vector.tensor_tensor(
                            out=oht4[:, 0:ng * P],
                            in0=iota_col.to_broadcast([P, ng * P]),
                            in1=pbc[:, 0:ng * P], op=AL.is_equal)
                        oh4 = sbp.tile([P, GRP * P], bf16, tag="oh4")
                        nc.vector.tensor_tensor(
                            out=oh4[:, 0:ng * P].rearrange("p (g j) -> p g j", g=ng),
                            in0=dstlocT_sb[:, gt:gt + ng]
                                .rearrange("p (g o) -> p g o", o=1)
                                .to_broadcast([P, ng, P]),
                            in1=iota_row.rearrange("p (o j) -> p o j", o=1)
                                .to_broadcast([P, ng, P]),
                            op=AL.is_equal)
                    grp = gt % GRP
                    g = gpo.tile([P, rowlen], f32, tag="g")
                    nc.gpsimd.indirect_dma_start(
                        out=g[:], out_offset=None, in_=tabsrc[:],
                        in_offset=bass.IndirectOffsetOnAxis(
                            ap=srcsb[:, gt:gt + 1], axis=0))
                    pD = psE.tile([P, 4], f32, tag="pD")
                    nc.tensor.matmul(out=pD[:, 0:nh],
                                     lhsT=oht4[:, grp * P:(grp + 1) * P],
                                     rhs=adst[:, b * nh:(b + 1) * nh],
                                     start=True, stop=True)
                    e4 = gpo.tile([P, 4], f32, tag="e4")
                    nc.vector.tensor_tensor(out=e4[:, 0:nh],
                                            in0=g[:, 64:64 + nh],
                                            in1=pD[:, 0:nh], op=AL.add)
                    # leaky_relu(x) = max(x, 0.2x); HW Lrelu table has fixed
                    # slope 0.01 and ignores alpha
                    t4 = gpo.tile([P, 4], f32, tag="t4")
                    nc.vector.tensor_scalar_mul(out=t4[:, 0:nh], in0=e4[:, 0:nh],
                                                scalar1=NEG)
                    l4 = gpo.tile([P, 4], f32, tag="l4")
                    nc.vector.tensor_tensor(out=l4[:, 0:nh], in0=e4[:, 0:nh],
                                            in1=t4[:, 0:nh], op=AL.max)
                    ex4 = gpo.tile([P, 4], f32, tag="ex4")
                    nc.scalar.activation(out=ex4[:, 0:nh], in_=l4[:, 0:nh],
                                         func=AF.Exp)
                    m = gpo.tile([P, 132], bf16, tag="m")
                    nc.vector.tensor_tensor(
                        out=m[:, 0:128].rearrange("p (h c) -> p h c", h=nh),
                        in0=g[:, 0:64].bitcast(bf16)
                            .rearrange("p (h c) -> p h c", h=nh),
                        in1=ex4[:, 0:nh].rearrange("p (h o) -> p h o", o=1)
                            .to_broadcast([P, nh, 128 // nh]),
                        op=AL.mult)
                    nc.vector.tensor_copy(out=m[:, 128:128 + nh], in_=ex4[:, 0:nh])
                    if first:
                        pb = psF.tile([P, mcols], f32, tag="pb")
                    nc.tensor.matmul(out=pb[:], lhsT=oh4[:, grp * P:(grp + 1) * P],
                                     rhs=m[:, 0:mcols], start=first, stop=last)
                    if last:
                        finalize(layer, b, pb)

            def finalize(layer, b, pb):
                rows = min(P, NPC - b * P)
                nh = HEADS if layer == 1 else 1
                den = gpo.tile([P, 4], f32, tag="den")
                nc.vector.tensor_scalar_add(out=den[:, 0:nh],
                                            in0=pb[:, 128:128 + nh],
                                            scalar1=1e-16)
                rec = gpo.tile([P, 4], f32, tag="rec")
                nc.vector.reciprocal(out=rec[:, 0:nh], in_=den[:, 0:nh])
                if layer == 1:
                    tmp = sbp.tile([P, 128], f32, tag="tmp")
                    nc.vector.tensor_tensor(
                        out=tmp[:].rearrange("p (h c) -> p h c", h=nh),
                        in0=pb[:, 0:128].rearrange("p (h c) -> p h c", h=nh),
                        in1=rec[:, 0:nh].rearrange("p (h o) -> p h o", o=1)
                            .to_broadcast([P, nh, 128 // nh]),
                        op=AL.mult)
                    nc.vector.tensor_tensor(out=tmp[:], in0=tmp[:], in1=b1bc[:],
                                            op=AL.add)
                    nc.vector.tensor_scalar_max(out=tmp[:], in0=tmp[:], scalar1=0.0)
                    upk = sbp.tile([P, ROW1], f32, tag="upk")  # ROW1>=ROW2
                    nc.vector.tensor_copy(out=upk[:, 0:64].bitcast(bf16), in_=tmp[:])
                    pt = psF.tile([P, P], f32, tag="pt")
                    nc.tensor.transpose(out=pt[:], in_=tmp[:], identity=ident[:])
                    uT = sbp.tile([P, P], bf16, tag="uT")
                    nc.vector.tensor_copy(out=uT[:], in_=pt[:])
                    pa2 = psE.tile([P, 2], f32, tag="pD")
                    nc.tensor.matmul(out=pa2[:], lhsT=uT[:], rhs=vsd[:],
                                     start=True, stop=True)
                    nc.vector.tensor_copy(out=upk[:, 64:65], in_=pa2[:, 0:1])
                    nc.vector.tensor_copy(out=adst2[:, b:b + 1], in_=pa2[:, 1:2])
                    nc.sync.dma_start(out=ltab2[b * P:b * P + rows, :],
                                      in_=upk[0:rows, 0:ROW2])
                else:
                    agg = sbp.tile([P, P], f32, tag="tmp")
                    nc.vector.tensor_scalar_mul(out=agg[:], in0=pb[:, 0:128],
                                                scalar1=rec[:, 0:1])
                    pt = psF.tile([P, P], f32, tag="pt")
                    nc.tensor.transpose(out=pt[:], in_=agg[:], identity=ident[:])
                    aT = sbp.tile([P, P], bf16, tag="uT")
                    nc.vector.tensor_copy(out=aT[:], in_=pt[:])
                    pz = psE.tile([P, 16], f32, tag="pD")
                    nc.tensor.matmul(out=pz[:], lhsT=aT[:], rhs=W2_sb,
                                     start=True, stop=True)
                    z = gpo.tile([P, 16], f32, tag="z")
                    nc.vector.tensor_tensor(out=z[:], in0=pz[:], in1=b2bc[:],
                                            op=AL.add)
                    mx = gpo.tile([P, 1], f32, tag="mx")
                    nc.vector.tensor_reduce(out=mx[:], in_=z[:],
                                            axis=mybir.AxisListType.X,
                                            op=AL.max, negate=True)  # -max
                    es = gpo.tile([P, 16], f32, tag="es")
                    ssum = gpo.tile([P, 1], f32, tag="ssum")
                    nc.scalar.activation(out=es[:], in_=z[:], func=AF.Exp,
                                         bias=mx[:], accum_out=ssum[:])
                    ls = gpo.tile([P, 1], f32, tag="ls")
                    nc.scalar.activation(out=ls[:], in_=ssum[:], func=AF.Ln)
                    sh = gpo.tile([P, 1], f32, tag="sh")
                    nc.vector.tensor_tensor(out=sh[:], in0=ls[:], in1=mx[:],
                                            op=AL.subtract)  # ln(s) - (-max)... see note
                    res = gpo.tile([P, 16], f32, tag="res")
                    nc.vector.tensor_scalar_sub(out=res[:], in0=z[:],
                                                scalar1=sh[:, 0:1])
                    nc.sync.dma_start(out=out_d[b * P:b * P + rows, :],
                                      in_=res[0:rows, :])

            with nc.named_scope("edge1"):
                edge_phase(1)

            with nc.named_scope("allgather"):
                nc.gpsimd.collective_compute(
                    "AllGather", mybir.AluOpType.bypass,
                    replica_groups=[list(range(C))],
                    ins=[ltab2[:]], outs=[tab2[:]])

            with nc.named_scope("edge2"):
                edge_phase(2)

    from wait_fix import split_excess_waits
    split_excess_waits(nc)
    return nc


# log_softmax shift note: out = z - max - ln(sum(exp(z - max))).
# mx holds -max (negate=True). es = exp(z + mx), ssum = sum(es), ls = ln(ssum).
# shift = max + ls = ls - mx. res = z - shift.


def _host_arrays(x, W1, att_src1, att_dst1, b1, W2, att_src2, att_dst2, b2,
                 srcT1, srcT2, dstloc, TT):
    TTg = ((TT + CHT - 1) // CHT) * CHT
    xT = np.zeros((P, XPAD), bfnp)
    xT[:, 0:N] = np.asarray(x, np.float32).T.astype(bfnp)

    x_own = np.zeros((C, P, PADN), bfnp)
    for c in range(C):
        end = min(c * NPC + PADN, XPAD)
        x_own[c, :, 0:end - c * NPC] = xT[:, c * NPC:end]

    iota_row = np.broadcast_to(np.arange(P, dtype=np.int32)[None, :], (P, P))
    iota_col = np.arange(P, dtype=np.int32).reshape(P, 1)
    idxs = np.zeros((C, P, 3 * TT + P + 1), np.int32)
    for c in range(C):
        idxs[c, :, 0:TT] = srcT1[c].T
        idxs[c, :, TT:2 * TT] = srcT2[c].T
        idxs[c, :, 2 * TT:3 * TT] = dstloc[c].T
        idxs[c, :, 3 * TT:3 * TT + P] = iota_row
        idxs[c, :, 3 * TT + P:] = iota_col

    dstrow = np.zeros((C, 1, TTg * P), bfnp)
    for c in range(C):
        dstrow[c, 0, 0:TT * P] = dstloc[c].reshape(-1).astype(bfnp)

    NCBF = 128 + 128 + 8 + 16 + 128 + 2 + 128
    cbf = np.zeros((P, NCBF), bfnp)
    W1f = np.asarray(W1, np.float32)
    cbf[:, 0:128] = W1f.astype(bfnp)
    cbf[:, 128:256] = W1f.T.astype(bfnp)
    ablk = np.zeros((128, 8), np.float32)
    for h in range(HEADS):
        ablk[h * HID:(h + 1) * HID, h] = np.asarray(att_src1, np.float32)[h]
        ablk[h * HID:(h + 1) * HID, 4 + h] = np.asarray(att_dst1, np.float32)[h]
    cbf[:, 256:264] = ablk.astype(bfnp)
    W2f = np.asarray(W2, np.float32)
    cbf[:, 264:280] = W2f.astype(bfnp)
    cbf[0:16, 280:408] = W2f.T.astype(bfnp)
    cbf[0:16, 408:409] = np.asarray(att_src2, np.float32).T.astype(bfnp)
    cbf[0:16, 409:410] = np.asarray(att_dst2, np.float32).T.astype(bfnp)
    cbf[0:1, 410:538] = np.ones((1, 128), bfnp)

    NCF = 128 + 16 + ROW1 + ROW2 + 128
    cf = np.zeros((1, NCF), np.float32)
    cf[0, 0:128] = np.asarray(b1, np.float32)
    cf[0, 128:144] = np.asarray(b2, np.float32)
    pad1 = np.zeros(ROW1, np.float32)
    pad1[64:68] = -1e9
    cf[0, 144:144 + ROW1] = pad1
    pad2 = np.zeros(ROW2, np.float32)
    pad2[64] = -1e9
    cf[0, 144 + ROW1:144 + ROW1 + ROW2] = pad2
    cf[0, 144 + ROW1 + ROW2:NCF] = 1.0

    return xT, x_own, idxs, dstrow, cbf, cf


_CACHE = {}


def kernel(x, edge_index, W1, att_src1, att_dst1, b1, W2, att_src2, att_dst2, b2,
           _trace=False, _tmpdir=None):
    srcT1, srcT2, dstloc, TB, TT, tile_starts = _prep(np.asarray(edge_index))
    xT, x_own, idxs, dstrow, cbf, cf = _host_arrays(
        x, W1, att_src1, att_dst1, b1, W2, att_src2, att_dst2, b2,
        srcT1, srcT2, dstloc, TT)

    key = (TT, tuple(int(t) for t in TB))
    if key not in _CACHE:
        _CACHE[key] = _build(TT, TB, tile_starts)
    nc = _CACHE[key]

    in_maps = []
    for c in range(C):
        in_maps.append({
            "x_t": xT, "x_own": np.ascontiguousarray(x_own[c]),
            "idxs": np.ascontiguousarray(idxs[c]),
            "dstrow": np.ascontiguousarray(dstrow[c]),
            "cbf": cbf, "cf": cf,
        })

    res = run_bass_kernel_spmd(nc, in_maps, list(range(C)), trace=_trace,
                               tmpdir=_tmpdir)
    out = np.concatenate([res.results[c]["out"] for c in range(C)], axis=0)
    kernel.last_results = res
    return out.astype(np.float32)


# revision 8
# speedup vs baseline: 1.4479x; 1.0258x over previous
"""GAT (2-layer, PyG-style) Trainium2 kernel, edge-parallel across 8 NeuronCores.

Self-contained: host-side numpy preprocessing (sharding / edge sorting / index
tables), Bass/Tile kernel build, SPMD execution on cores 0-7, gather of the
full [50000, 16] log-softmax output.

Strategy:
  - edges (plus self loops) sorted by dst; dst-range sharded: core d owns dst in
    [d*6250, (d+1)*6250) so all segment reductions are core-local.
  - per 128-node block, edges are tiled into 128-edge tiles; segment softmax
    sums are one-hot matmuls accumulated in PSUM (no scatter).
  - softmax without max subtraction (logits are O(1) for this model; verified
    offline: max |e| < 4, so exp never overflows; self loops keep denom > 0).
  - layer-2 messages: W2 is linear and applied after aggregation (heads=1), so
    aggregate relu(h1) with layer-2 attention weights first, then matmul W2.
  - node tables replicated: each core computes the full [N] layer-1 table; the
    layer-2 table is AllGathered (each core produces its own 6250 rows).
"""
import numpy as np
import ml_dtypes

import concourse.bass as bass
import concourse.mybir as mybir
import concourse.tile as tile
from concourse.bass_utils import run_bass_kernel_spmd

bfnp = ml_dtypes.bfloat16

N = 50000
E = 800000
IN_CH = 128
HID = 32
HEADS = 4
OUT_CH = 16
NEG = 0.2
C = 8
NPC = N // C               # 6250 nodes per core
P = 128
NB = (NPC + P - 1) // P    # 49 blocks per core
PADN = NB * P              # 6272 rows per core stripe in allgathered table
ROW1 = 68                  # tab1 row: 64 f32 words (128 bf16 h) + 4 f32 a_src
ROW2 = 65                  # tab2 row: 64 f32 words (128 bf16 u) + 1 f32 a_src2
GRP = 4                    # one-hot generation group (one 512-wide psum bank)
CHT = 64                   # dstrow chunk length in tiles
NT1 = (N + P - 1) // P     # 391 node tiles (50048 padded)
XPAD = NT1 * P

f32 = mybir.dt.float32
bf16 = mybir.dt.bfloat16
i32 = mybir.dt.int32


def _prep(edge_index):
    """Sort/shard/tile edges. Returns per-core index arrays + block tile counts."""
    src = np.concatenate([np.asarray(edge_index[0]), np.arange(N, dtype=np.int64)])
    dst = np.concatenate([np.asarray(edge_index[1]), np.arange(N, dtype=np.int64)])
    order = np.argsort(dst, kind="stable")
    src = src[order].astype(np.int64)
    dst = dst[order].astype(np.int64)

    core_of = dst // NPC
    per_core = []
    counts = np.zeros((C, NB), np.int64)
    for c in range(C):
        m = core_of == c
        s, d = src[m], dst[m]
        loc = d - c * NPC
        blk = loc // P
        np.add.at(counts[c], blk, 1)
        per_core.append((s, loc, blk))

    TB = np.maximum(1, (counts.max(axis=0) + P - 1) // P)
    TT = int(TB.sum())
    tile_starts = np.zeros(NB + 1, np.int64)
    tile_starts[1:] = np.cumsum(TB)

    srcT1 = np.zeros((C, TT, P), np.int32)
    srcT2 = np.zeros((C, TT, P), np.int32)
    dstloc = np.zeros((C, TT, P), np.int32)

    for c in range(C):
        s, loc, blk = per_core[c]
        for b in range(NB):
            m = blk == b
            sb_, lb = s[m], loc[m] - b * P
            n = len(sb_)
            cap = int(TB[b]) * P
            s1 = np.full(cap, N, np.int64)          # tab1 pad row (a_src=-1e9)
            s1[:n] = sb_
            own = sb_ // NPC
            s2 = np.full(cap, NPC, np.int64)        # core0 stripe pad row
            s2[:n] = own * PADN + (sb_ - own * NPC)
            dl = np.zeros(cap, np.int64)            # pad edges hit col 0, ex=0
            dl[:n] = lb
            t0 = tile_starts[b]
            srcT1[c, t0:t0 + TB[b]] = s1.reshape(int(TB[b]), P)
            srcT2[c, t0:t0 + TB[b]] = s2.reshape(int(TB[b]), P)
            dstloc[c, t0:t0 + TB[b]] = dl.reshape(int(TB[b]), P)

    return srcT1, srcT2, dstloc, TB, TT, tile_starts


def _build(TT, TB, tile_starts):
    """Build the SPMD Bass graph (identical for all cores)."""
    TTg = ((TT + CHT - 1) // CHT) * CHT
    NCBF = 128 + 128 + 8 + 16 + 128 + 2 + 128
    NCF = 128 + 16 + ROW1 + ROW2 + 128
    tsl = [int(t) for t in tile_starts]
    blk_of_tile = np.zeros(TT, np.int64)
    for b in range(NB):
        blk_of_tile[tsl[b]:tsl[b + 1]] = b

    nc = bass.Bass()
    x_t = nc.declare_dram_parameter("x_t", [P, XPAD], bf16, isOutput=False)
    x_own = nc.declare_dram_parameter("x_own", [P, PADN], bf16, isOutput=False)
    idxs = nc.declare_dram_parameter("idxs", [P, 3 * TT + P + 1], i32, isOutput=False)
    dstrow = nc.declare_dram_parameter("dstrow", [1, TTg * P], bf16, isOutput=False)
    cbf = nc.declare_dram_parameter("cbf", [P, NCBF], bf16, isOutput=False)
    cf = nc.declare_dram_parameter("cf", [1, NCF], f32, isOutput=False)
    out_d = nc.declare_dram_parameter("out", [NPC, OUT_CH], f32, isOutput=True)

    tab1 = nc.dram_tensor("tab1", [N + 1, ROW1], f32)
    ltab2 = nc.dram_tensor("ltab2", [PADN, ROW2], f32)
    tab2 = nc.dram_tensor("tab2", [C * PADN, ROW2], f32, addr_space="Shared")

    AL = mybir.AluOpType
    AF = mybir.ActivationFunctionType

    with tile.TileContext(nc) as tc:
        from contextlib import ExitStack
        with ExitStack() as ctx:
            cpool = ctx.enter_context(tc.tile_pool(name="const", bufs=1))

            # ---- constants ----
            ci = cpool.tile([P, 3 * TT + P + 1], i32)
            nc.sync.dma_start(out=ci[:], in_=idxs[:])
            srcT1_sb = ci[:, 0:TT]
            srcT2_sb = ci[:, TT:2 * TT]
            dstlocT_sb = ci[:, 2 * TT:3 * TT]
            iota_row = ci[:, 3 * TT:3 * TT + P]
            iota_col = ci[:, 3 * TT + P:3 * TT + P + 1]

            cb = cpool.tile([P, NCBF], bf16)
            nc.sync.dma_start(out=cb[:], in_=cbf[:])
            W1_sb = cb[:, 0:128]
            W1T_sb = cb[:, 128:256]
            ablk_sb = cb[:, 256:264]
            W2_sb = cb[:, 264:280]
            W2T_sb = cb[0:16, 280:408]
            att2T_sb = cb[0:16, 408:410]
            ones_bf = cb[0:1, 410:538]  # ones row [1, 128]

            cfs = cpool.tile([1, NCF], f32)
            nc.sync.dma_start(out=cfs[:], in_=cf[:])
            b1_row = cfs[:, 0:128]
            b2_row = cfs[:, 128:144]
            pad1_row = cfs[:, 144:144 + ROW1]
            pad2_row = cfs[:, 144 + ROW1:144 + ROW1 + ROW2]
            ones_f = cfs[:, 144 + ROW1 + ROW2:NCF]  # ones row [1, 128]

            xo = cpool.tile([P, PADN], bf16)
            nc.sync.dma_start(out=xo[:], in_=x_own[:])

            # prime DVE on const DMAs (keeps later waits <= 1 per instruction)
            pr = cpool.tile([P, 1], f32)
            nc.vector.tensor_tensor(out=pr[:], in0=ci[:, 0:1], in1=cb[:, 0:1],
                                    op=AL.add)
            nc.vector.tensor_tensor(out=pr[0:1, :], in0=cfs[0:1, 0:1],
                                    in1=xo[0:1, 0:1], op=AL.add)

            adst1 = cpool.tile([P, NB * HEADS], bf16)
            adst2 = cpool.tile([P, NB], bf16)

            from concourse.masks import make_identity
            ident = cpool.tile([P, P], f32)
            make_identity(nc, ident[:])

            with tc.tile_pool(name="ps0", bufs=2, space="PSUM") as ps0:
                # W1cat = [W1 | W1@ablk_src | W1@ablk_dst] bf16
                w1cat = cpool.tile([P, 136], bf16)
                nc.vector.tensor_copy(out=w1cat[:, 0:128], in_=W1_sb)
                ps_w = ps0.tile([P, 8], f32, tag="pw")
                nc.tensor.matmul(out=ps_w[:], lhsT=W1T_sb, rhs=ablk_sb,
                                 start=True, stop=True)
                nc.vector.tensor_copy(out=w1cat[:, 128:136], in_=ps_w[:])

                # vsd = W2 @ [att_src2.T | att_dst2.T] -> [128, 2] bf16
                vsd = cpool.tile([P, 2], bf16)
                ps_v = ps0.tile([P, 2], f32, tag="pw")
                nc.tensor.matmul(out=ps_v[:], lhsT=W2T_sb, rhs=att2T_sb,
                                 start=True, stop=True)
                nc.vector.tensor_copy(out=vsd[:], in_=ps_v[:])

                # bias broadcast rows -> [128, *] f32
                b1bc = cpool.tile([P, 128], f32)
                ps_b = ps0.tile([P, 128], f32, tag="pw")
                nc.tensor.matmul(out=ps_b[:], lhsT=ones_f, rhs=b1_row,
                                 start=True, stop=True)
                nc.vector.tensor_copy(out=b1bc[:], in_=ps_b[:])
                b2bc = cpool.tile([P, 16], f32)
                ps_b2 = ps0.tile([P, 16], f32, tag="pw")
                nc.tensor.matmul(out=ps_b2[:], lhsT=ones_f, rhs=b2_row,
                                 start=True, stop=True)
                nc.vector.tensor_copy(out=b2bc[:], in_=ps_b2[:])

                # pad rows
                nc.sync.dma_start(out=tab1[N:N + 1, :], in_=pad1_row)
                # only row NPC is ever gathered (the pad target); rows
                # NPC+1..PADN are never referenced
                nc.sync.dma_start(out=ltab2[NPC:NPC + 1, :], in_=pad2_row)

                # ---- phase 1: full h1/a_src1 table; a_dst1 for own nodes ----
                with nc.named_scope("phase1"), \
                     tc.tile_pool(name="p1", bufs=4) as p1:
                    XC = 16
                    for c0 in range(0, NT1, XC):
                        nct = min(XC, NT1 - c0)
                        xc = p1.tile([P, XC * P], bf16, tag="xc")
                        nc.sync.dma_start(out=xc[:, 0:nct * P],
                                          in_=x_t[:, c0 * P:(c0 + nct) * P])
                        for k in range(nct):
                            ph = ps0.tile([P, 136], f32, tag="ph")
                            nc.tensor.matmul(out=ph[:],
                                             lhsT=xc[:, k * P:(k + 1) * P],
                                             rhs=w1cat[:], start=True, stop=True)
                            pk = p1.tile([P, ROW1], f32, tag="pk")
                            nc.vector.tensor_copy(out=pk[:, 0:64].bitcast(bf16),
                                                  in_=ph[:, 0:128])
                            nc.vector.tensor_copy(out=pk[:, 64:68],
                                                  in_=ph[:, 128:132])
                            nt = c0 + k
                            nr = min(P, N - nt * P)
                            nc.sync.dma_start(out=tab1[nt * P:nt * P + nr, :],
                                              in_=pk[0:nr, :])
                    for b in range(NB):
                        pa = ps0.tile([P, 4], f32, tag="pa")
                        nc.tensor.matmul(out=pa[:],
                                         lhsT=xo[:, b * P:(b + 1) * P],
                                         rhs=w1cat[:, 132:136],
                                         start=True, stop=True)
                        nc.vector.tensor_copy(out=adst1[:, b * 4:(b + 1) * 4],
                                              in_=pa[:])

            # ---- edge phases ----
            sbp = ctx.enter_context(tc.tile_pool(name="sbp", bufs=4))
            gpo = ctx.enter_context(tc.tile_pool(name="gpo", bufs=8))
            drp = ctx.enter_context(tc.tile_pool(name="drp", bufs=2))
            psE = ctx.enter_context(tc.tile_pool(name="psE", bufs=2, space="PSUM"))
            psF = ctx.enter_context(tc.tile_pool(name="psF", bufs=2, space="PSUM"))

            def edge_phase(layer):
                tabsrc = tab1 if layer == 1 else tab2
                srcsb = srcT1_sb if layer == 1 else srcT2_sb
                rowlen = ROW1 if layer == 1 else ROW2
                nh = HEADS if layer == 1 else 1
                mcols = 132 if layer == 1 else 129
                adst = adst1 if layer == 1 else adst2
                drow_t = None
                pb = None
                for g0 in range(0, TT, GRP):
                    ng = min(GRP, TT - g0)
                    if g0 % CHT == 0:
                        drow_t = drp.tile([1, CHT * P], bf16, tag="drow")
                        nc.sync.dma_start(out=drow_t[:],
                                          in_=dstrow[:, g0 * P:(g0 + CHT) * P])
                    off = (g0 % CHT) * P
                    pbc = psE.tile([P, GRP * P], f32, tag="pbc")
                    nc.tensor.matmul(out=pbc[:, 0:ng * P], lhsT=ones_bf,
                                     rhs=drow_t[:, off:off + ng * P],
                                     start=True, stop=True)
                    oht4 = sbp.tile([P, GRP * P], bf16, tag="oht4")
                    nc.vector.tensor_tensor(
                        out=oht4[:, 0:ng * P],
                        in0=iota_col.to_broadcast([P, ng * P]),
                        in1=pbc[:, 0:ng * P], op=AL.is_equal)
                    oh4 = sbp.tile([P, GRP * P], bf16, tag="oh4")
                    nc.vector.tensor_tensor(
                        out=oh4[:, 0:ng * P].rearrange("p (g j) -> p g j", g=ng),
                        in0=dstlocT_sb[:, g0:g0 + ng]
                            .rearrange("p (g o) -> p g o", o=1)
                            .to_broadcast([P, ng, P]),
                        in1=iota_row.rearrange("p (o j) -> p o j", o=1)
                            .to_broadcast([P, ng, P]),
                        op=AL.is_equal)
                    # gathers for the whole group into one buffer
                    g4 = gpo.tile([P, GRP * rowlen], f32, tag="g4")
                    for j in range(ng):
                        nc.gpsimd.indirect_dma_start(
                            out=g4[:, j * rowlen:(j + 1) * rowlen],
                            out_offset=None, in_=tabsrc[:],
                            in_offset=bass.IndirectOffsetOnAxis(
                                ap=srcsb[:, g0 + j:g0 + j + 1], axis=0))
                    # a_dst per edge via one-hot matmuls (bf16)
                    pD16 = psE.tile([P, GRP * 4], f32, tag="pD")
                    for j in range(ng):
                        bj = int(blk_of_tile[g0 + j])
                        nc.tensor.matmul(out=pD16[:, j * 4:j * 4 + nh],
                                         lhsT=oht4[:, j * P:(j + 1) * P],
                                         rhs=adst[:, bj * nh:(bj + 1) * nh],
                                         start=True, stop=True)
                    # e = a_src + a_dst, leaky, exp -- batched over the group
                    e16 = gpo.tile([P, GRP * 4], f32, tag="e16")
                    nc.vector.tensor_tensor(
                        out=e16[:, 0:ng * 4].rearrange("p (g h) -> p g h", g=ng)[:, :, 0:nh],
                        in0=g4[:, 0:ng * rowlen]
                            .rearrange("p (g r) -> p g r", g=ng)[:, :, 64:64 + nh],
                        in1=pD16[:, 0:ng * 4]
                            .rearrange("p (g h) -> p g h", g=ng)[:, :, 0:nh],
                        op=AL.add)
                    t16 = gpo.tile([P, GRP * 4], f32, tag="t16")
                    nc.vector.tensor_scalar_mul(out=t16[:, 0:ng * 4],
                                                in0=e16[:, 0:ng * 4], scalar1=NEG)
                    l16 = gpo.tile([P, GRP * 4], f32, tag="l16")
                    nc.vector.tensor_tensor(out=l16[:, 0:ng * 4],
                                            in0=e16[:, 0:ng * 4],
                                            in1=t16[:, 0:ng * 4], op=AL.max)
                    m4 = gpo.tile([P, GRP * 132], bf16, tag="m4")
                    nc.scalar.activation(
                        out=m4[:, 0:ng * 132]
                            .rearrange("p (g r) -> p g r", g=ng)[:, :, 128:132],
                        in_=l16[:, 0:ng * 4].rearrange("p (g h) -> p g h", g=ng),
                        func=AF.Exp)
                    g4b = g4[:, 0:ng * rowlen].bitcast(bf16)   # [P, ng*rowlen*2]
                    for j in range(ng):
                        nc.vector.tensor_tensor(
                            out=m4[:, j * 132:j * 132 + 128]
                                .rearrange("p (h c) -> p h c", h=nh),
                            in0=g4b[:, j * rowlen * 2:j * rowlen * 2 + 128]
                                .rearrange("p (h c) -> p h c", h=nh),
                            in1=m4[:, j * 132 + 128:j * 132 + 128 + nh]
                                .rearrange("p (h o) -> p h o", o=1)
                                .to_broadcast([P, nh, 128 // nh]),
                            op=AL.mult)
                    for j in range(ng):
                        gt = g0 + j
                        b = int(blk_of_tile[gt])
                        first = gt == tsl[b]
                        last = gt == tsl[b + 1] - 1
                        if first:
                            pb = psF.tile([P, mcols], f32, tag="pb")
                        nc.tensor.matmul(out=pb[:],
                                         lhsT=oh4[:, j * P:(j + 1) * P],
                                         rhs=m4[:, j * 132:j * 132 + mcols],
                                         start=first, stop=last)
                        if last:
                            finalize(layer, b, pb)

            def finalize(layer, b, pb):
                rows = min(P, NPC - b * P)
                nh = HEADS if layer == 1 else 1
                den = gpo.tile([P, 4], f32, tag="den")
                nc.vector.tensor_scalar_add(out=den[:, 0:nh],
                                            in0=pb[:, 128:128 + nh],
                                            scalar1=1e-16)
                rec = gpo.tile([P, 4], f32, tag="rec")
                nc.vector.reciprocal(out=rec[:, 0:nh], in_=den[:, 0:nh])
                if layer == 1:
                    tmp = sbp.tile([P, 128], f32, tag="tmp")
                    nc.vector.tensor_tensor(
                        out=tmp[:].rearrange("p (h c) -> p h c", h=nh),
                        in0=pb[:, 0:128].rearrange("p (h c) -> p h c", h=nh),
                        in1=rec[:, 0:nh].rearrange("p (h o) -> p h o", o=1)
                            .to_broadcast([P, nh, 128 // nh]),
                        op=AL.mult)
                    nc.vector.tensor_tensor(out=tmp[:], in0=tmp[:], in1=b1bc[:],
                                            op=AL.add)
                    nc.vector.tensor_scalar_max(out=tmp[:], in0=tmp[:], scalar1=0.0)
                    upk = sbp.tile([P, ROW1], f32, tag="upk")  # ROW1>=ROW2
                    nc.vector.tensor_copy(out=upk[:, 0:64].bitcast(bf16), in_=tmp[:])
                    pt = psF.tile([P, P], f32, tag="pt")
                    nc.tensor.transpose(out=pt[:], in_=tmp[:], identity=ident[:])
                    uT = sbp.tile([P, P], bf16, tag="uT")
                    nc.vector.tensor_copy(out=uT[:], in_=pt[:])
                    pa2 = psE.tile([P, 2], f32, tag="pD")
                    nc.tensor.matmul(out=pa2[:], lhsT=uT[:], rhs=vsd[:],
                                     start=True, stop=True)
                    nc.vector.tensor_copy(out=upk[:, 64:65], in_=pa2[:, 0:1])
                    nc.vector.tensor_copy(out=adst2[:, b:b + 1], in_=pa2[:, 1:2])
                    nc.sync.dma_start(out=ltab2[b * P:b * P + rows, :],
                                      in_=upk[0:rows, 0:ROW2])
                else:
                    agg = sbp.tile([P, P], f32, tag="tmp")
                    nc.vector.tensor_scalar_mul(out=agg[:], in0=pb[:, 0:128],
                                                scalar1=rec[:, 0:1])
                    pt = psF.tile([P, P], f32, tag="pt")
                    nc.tensor.transpose(out=pt[:], in_=agg[:], identity=ident[:])
                    aT = sbp.tile([P, P], bf16, tag="uT")
                    nc.vector.tensor_copy(out=aT[:], in_=pt[:])
                    pz = psE.tile([P, 16], f32, tag="pD")
                    nc.tensor.matmul(out=pz[:], lhsT=aT[:], rhs=W2_sb,
                                     start=True, stop=True)
                    z = gpo.tile([P, 16], f32, tag="z")
                    nc.vector.tensor_tensor(out=z[:], in0=pz[:], in1=b2bc[:],
                                            op=AL.add)
                    mx = gpo.tile([P, 1], f32, tag="mx")
                    nc.vector.tensor_reduce(out=mx[:], in_=z[:],
                                            axis=mybir.AxisListType.X,
                                            op=AL.max, negate=True)  # -max
                    es = gpo.tile([P, 16], f32, tag="es")
                    ssum = gpo.tile([P, 1], f32, tag="ssum")
                    nc.scalar.activation(out=es[:], in_=z[:], func=AF.Exp,
                                         bias=mx[:], accum_out=ssum[:])
                    ls = gpo.tile([P, 1], f32, tag="ls")
                    nc.scalar.activation(out=ls[:], in_=ssum[:], func=AF.Ln)
                    sh = gpo.tile([P, 1], f32, tag="sh")
                    nc.vector.tensor_tensor(out=sh[:], in0=ls[:], in1=mx[:],
                                            op=AL.subtract)  # ln(s) - (-max)... see note
                    res = gpo.tile([P, 16], f32, tag="res")
                    nc.vector.tensor_scalar_sub(out=res[:], in0=z[:],
                                                scalar1=sh[:, 0:1])
                    nc.sync.dma_start(out=out_d[b * P:b * P + rows, :],
                                      in_=res[0:rows, :])

            with nc.named_scope("edge1"):
                edge_phase(1)

            with nc.named_scope("allgather"):
                nc.gpsimd.collective_compute(
                    "AllGather", mybir.AluOpType.bypass,
                    replica_groups=[list(range(C))],
                    ins=[ltab2[:]], outs=[tab2[:]])

            with nc.named_scope("edge2"):
                edge_phase(2)

    from wait_fix import split_excess_waits
    split_excess_waits(nc)
    return nc


# log_softmax shift note: out = z - max - ln(sum(exp(z - max))).
# mx holds -max (negate=True). es = exp(z + mx), ssum = sum(es), ls = ln(ssum).
# shift = max + ls = ls - mx. res = z - shift.


def _host_arrays(x, W1, att_src1, att_dst1, b1, W2, att_src2, att_dst2, b2,
                 srcT1, srcT2, dstloc, TT):
    TTg = ((TT + CHT - 1) // CHT) * CHT
    xT = np.zeros((P, XPAD), bfnp)
    xT[:, 0:N] = np.asarray(x, np.float32).T.astype(bfnp)

    x_own = np.zeros((C, P, PADN), bfnp)
    for c in range(C):
        end = min(c * NPC + PADN, XPAD)
        x_own[c, :, 0:end - c * NPC] = xT[:, c * NPC:end]

    iota_row = np.broadcast_to(np.arange(P, dtype=np.int32)[None, :], (P, P))
    iota_col = np.arange(P, dtype=np.int32).reshape(P, 1)
    idxs = np.zeros((C, P, 3 * TT + P + 1), np.int32)
    for c in range(C):
        idxs[c, :, 0:TT] = srcT1[c].T
        idxs[c, :, TT:2 * TT] = srcT2[c].T
        idxs[c, :, 2 * TT:3 * TT] = dstloc[c].T
        idxs[c, :, 3 * TT:3 * TT + P] = iota_row
        idxs[c, :, 3 * TT + P:] = iota_col

    dstrow = np.zeros((C, 1, TTg * P), bfnp)
    for c in range(C):
        dstrow[c, 0, 0:TT * P] = dstloc[c].reshape(-1).astype(bfnp)

    NCBF = 128 + 128 + 8 + 16 + 128 + 2 + 128
    cbf = np.zeros((P, NCBF), bfnp)
    W1f = np.asarray(W1, np.float32)
    cbf[:, 0:128] = W1f.astype(bfnp)
    cbf[:, 128:256] = W1f.T.astype(bfnp)
    ablk = np.zeros((128, 8), np.float32)
    for h in range(HEADS):
        ablk[h * HID:(h + 1) * HID, h] = np.asarray(att_src1, np.float32)[h]
        ablk[h * HID:(h + 1) * HID, 4 + h] = np.asarray(att_dst1, np.float32)[h]
    cbf[:, 256:264] = ablk.astype(bfnp)
    W2f = np.asarray(W2, np.float32)
    cbf[:, 264:280] = W2f.astype(bfnp)
    cbf[0:16, 280:408] = W2f.T.astype(bfnp)
    cbf[0:16, 408:409] = np.asarray(att_src2, np.float32).T.astype(bfnp)
    cbf[0:16, 409:410] = np.asarray(att_dst2, np.float32).T.astype(bfnp)
    cbf[0:1, 410:538] = np.ones((1, 128), bfnp)

    NCF = 128 + 16 + ROW1 + ROW2 + 128
    cf = np.zeros((1, NCF), np.float32)
    cf[0, 0:128] = np.asarray(b1, np.float32)
    cf[0, 128:144] = np.asarray(b2, np.float32)
    pad1 = np.zeros(ROW1, np.float32)
    pad1[64:68] = -1e9
    cf[0, 144:144 + ROW1] = pad1
    pad2 = np.zeros(ROW2, np.float32)
    pad2[64] = -1e9
    cf[0, 144 + ROW1:144 + ROW1 + ROW2] = pad2
    cf[0, 144 + ROW1 + ROW2:NCF] = 1.0

    return xT, x_own, idxs, dstrow, cbf, cf


_CACHE = {}


def kernel(x, edge_index, W1, att_src1, att_dst1, b1, W2, att_src2, att_dst2, b2,
           _trace=False, _tmpdir=None):
    srcT1, srcT2, dstloc, TB, TT, tile_starts = _prep(np.asarray(edge_index))
    xT, x_own, idxs, dstrow, cbf, cf = _host_arrays(
        x, W1, att_src1, att_dst1, b1, W2, att_src2, att_dst2, b2,
        srcT1, srcT2, dstloc, TT)

    key = (TT, tuple(int(t) for t in TB))
    if key not in _CACHE:
        _CACHE[key] = _build(TT, TB, tile_starts)
    nc = _CACHE[key]

    in_maps = []
    for c in range(C):
        in_maps.append({
            "x_t": xT, "x_own": np.ascontiguousarray(x_own[c]),
            "idxs": np.ascontiguousarray(idxs[c]),
            "dstrow": np.ascontiguousarray(dstrow[c]),
            "cbf": cbf, "cf": cf,
        })

    res = run_bass_kernel_spmd(nc, in_maps, list(range(C)), trace=_trace,
                               tmpdir=_tmpdir)
    out = np.concatenate([res.results[c]["out"] for c in range(C)], axis=0)
    kernel.last_results = res
    return out.astype(np.float32)


# revision 10
# speedup vs baseline: 1.4645x; 1.0114x over previous
"""GAT (2-layer, PyG-style) Trainium2 kernel, edge-parallel across 8 NeuronCores.

Self-contained: host-side numpy preprocessing (sharding / edge sorting / index
tables), Bass/Tile kernel build, SPMD execution on cores 0-7, gather of the
full [50000, 16] log-softmax output.

Strategy:
  - edges (plus self loops) sorted by dst; dst-range sharded: core d owns dst in
    [d*6250, (d+1)*6250) so all segment reductions are core-local.
  - per 128-node block, edges are tiled into 128-edge tiles; segment softmax
    sums are one-hot matmuls accumulated in PSUM (no scatter).
  - softmax without max subtraction (logits are O(1) for this model; verified
    offline: max |e| < 4, so exp never overflows; self loops keep denom > 0).
  - layer-2 messages: W2 is linear and applied after aggregation (heads=1), so
    aggregate relu(h1) with layer-2 attention weights first, then matmul W2.
  - node tables replicated: each core computes the full [N] layer-1 table; the
    layer-2 table is AllGathered (each core produces its own 6250 rows).
"""
import numpy as np
import ml_dtypes

import concourse.bass as bass
import concourse.mybir as mybir
import concourse.tile as tile
from concourse.bass_utils import run_bass_kernel_spmd

bfnp = ml_dtypes.bfloat16

N = 50000
E = 800000
IN_CH = 128
HID = 32
HEADS = 4
OUT_CH = 16
NEG = 0.2
C = 8
NPC = N // C               # 6250 nodes per core
P = 128
NB = (NPC + P - 1) // P    # 49 blocks per core
PADN = NB * P              # 6272 rows per core stripe in allgathered table
ROW1 = 68                  # tab1 row: 64 f32 words (128 bf16 h) + 4 f32 a_src
ROW2 = 65                  # tab2 row: 64 f32 words (128 bf16 u) + 1 f32 a_src2
GRP = 4                    # one-hot generation group (one 512-wide psum bank)
CHT = 64                   # dstrow chunk length in tiles
NT1 = (N + P - 1) // P     # 391 node tiles (50048 padded)
XPAD = NT1 * P

f32 = mybir.dt.float32
bf16 = mybir.dt.bfloat16
i32 = mybir.dt.int32


def _prep(edge_index):
    """Sort/shard/tile edges. Returns per-core index arrays + block tile counts."""
    src = np.concatenate([np.asarray(edge_index[0]), np.arange(N, dtype=np.int64)])
    dst = np.concatenate([np.asarray(edge_index[1]), np.arange(N, dtype=np.int64)])
    order = np.argsort(dst, kind="stable")
    src = src[order].astype(np.int64)
    dst = dst[order].astype(np.int64)

    core_of = dst // NPC
    per_core = []
    counts = np.zeros((C, NB), np.int64)
    for c in range(C):
        m = core_of == c
        s, d = src[m], dst[m]
        loc = d - c * NPC
        blk = loc // P
        np.add.at(counts[c], blk, 1)
        per_core.append((s, loc, blk))

    TB = np.maximum(1, (counts.max(axis=0) + P - 1) // P)
    TT = int(TB.sum())
    tile_starts = np.zeros(NB + 1, np.int64)
    tile_starts[1:] = np.cumsum(TB)

    srcT1 = np.zeros((C, TT, P), np.int32)
    srcT2 = np.zeros((C, TT, P), np.int32)
    dstloc = np.zeros((C, TT, P), np.int32)

    for c in range(C):
        s, loc, blk = per_core[c]
        for b in range(NB):
            m = blk == b
            sb_, lb = s[m], loc[m] - b * P
            n = len(sb_)
            cap = int(TB[b]) * P
            s1 = np.full(cap, N, np.int64)          # tab1 pad row (a_src=-1e9)
            s1[:n] = sb_
            own = sb_ // NPC
            ln = sb_ - own * NPC
            # chunked allgather layout: chunk k = ln//896 holds 8 stripes of 896
            s2 = np.full(cap, 6 * 7168 + 874, np.int64)   # core0 pad row (ln=6250)
            s2[:n] = (ln // 896) * 7168 + own * 896 + (ln % 896)
            dl = np.zeros(cap, np.int64)            # pad edges hit col 0, ex=0
            dl[:n] = lb
            t0 = tile_starts[b]
            srcT1[c, t0:t0 + TB[b]] = s1.reshape(int(TB[b]), P)
            srcT2[c, t0:t0 + TB[b]] = s2.reshape(int(TB[b]), P)
            dstloc[c, t0:t0 + TB[b]] = dl.reshape(int(TB[b]), P)

    return srcT1, srcT2, dstloc, TB, TT, tile_starts


def _build(TT, TB, tile_starts):
    """Build the SPMD Bass graph (identical for all cores)."""
    TTg = ((TT + CHT - 1) // CHT) * CHT
    NCBF = 128 + 128 + 8 + 16 + 128 + 2 + 128 + TT + 128
    NCF = 128 + 16 + ROW1 + ROW2 + 128
    tsl = [int(t) for t in tile_starts]
    blk_of_tile = np.zeros(TT, np.int64)
    for b in range(NB):
        blk_of_tile[tsl[b]:tsl[b + 1]] = b

    nc = bass.Bass()
    x_t = nc.declare_dram_parameter("x_t", [P, XPAD], bf16, isOutput=False)
    x_own = nc.declare_dram_parameter("x_own", [P, PADN], bf16, isOutput=False)
    idxs = nc.declare_dram_parameter("idxs", [P, 3 * TT + P + 1], i32, isOutput=False)
    dstrow = nc.declare_dram_parameter("dstrow", [1, TTg * P], bf16, isOutput=False)
    cbf = nc.declare_dram_parameter("cbf", [P, NCBF], bf16, isOutput=False)
    cf = nc.declare_dram_parameter("cf", [1, NCF], f32, isOutput=False)
    out_d = nc.declare_dram_parameter("out", [NPC, OUT_CH], f32, isOutput=True)

    tab1 = nc.dram_tensor("tab1", [N + 1, ROW1], f32)
    ltab2 = nc.dram_tensor("ltab2", [PADN, ROW2], f32)
    tab2 = nc.dram_tensor("tab2", [C * PADN, ROW2], f32, addr_space="Shared")

    AL = mybir.AluOpType
    AF = mybir.ActivationFunctionType

    with tile.TileContext(nc) as tc:
        from contextlib import ExitStack
        with ExitStack() as ctx:
            cpool = ctx.enter_context(tc.tile_pool(name="const", bufs=1))

            # ---- constants ----
            ci = cpool.tile([P, 3 * TT + P + 1], i32)
            nc.sync.dma_start(out=ci[:], in_=idxs[:])
            srcT1_sb = ci[:, 0:TT]
            srcT2_sb = ci[:, TT:2 * TT]
            dstlocT_sb = ci[:, 2 * TT:3 * TT]
            iota_row = ci[:, 3 * TT:3 * TT + P]
            iota_col = ci[:, 3 * TT + P:3 * TT + P + 1]

            cb = cpool.tile([P, NCBF], bf16)
            nc.sync.dma_start(out=cb[:], in_=cbf[:])
            W1_sb = cb[:, 0:128]
            W1T_sb = cb[:, 128:256]
            ablk_sb = cb[:, 256:264]
            W2_sb = cb[:, 264:280]
            W2T_sb = cb[0:16, 280:408]
            att2T_sb = cb[0:16, 408:410]
            ones_bf = cb[0:1, 410:538]  # ones row [1, 128]
            dstloc_bf = cb[:, 538:538 + TT]
            iota_row_bf = cb[:, 538 + TT:538 + TT + P]

            cfs = cpool.tile([1, NCF], f32)
            nc.sync.dma_start(out=cfs[:], in_=cf[:])
            b1_row = cfs[:, 0:128]
            b2_row = cfs[:, 128:144]
            pad1_row = cfs[:, 144:144 + ROW1]
            pad2_row = cfs[:, 144 + ROW1:144 + ROW1 + ROW2]
            ones_f = cfs[:, 144 + ROW1 + ROW2:NCF]  # ones row [1, 128]

            xo = cpool.tile([P, PADN], bf16)
            nc.sync.dma_start(out=xo[:], in_=x_own[:])

            # prime DVE on const DMAs (keeps later waits <= 1 per instruction)
            pr = cpool.tile([P, 1], f32)
            nc.vector.tensor_tensor(out=pr[:], in0=ci[:, 0:1], in1=cb[:, 0:1],
                                    op=AL.add)
            nc.vector.tensor_tensor(out=pr[0:1, :], in0=cfs[0:1, 0:1],
                                    in1=xo[0:1, 0:1], op=AL.add)

            adst1 = cpool.tile([P, NB * HEADS], bf16)
            adst2 = cpool.tile([P, NB], bf16)

            from concourse.masks import make_identity
            ident = cpool.tile([P, P], f32)
            make_identity(nc, ident[:])

            with tc.tile_pool(name="ps0", bufs=2, space="PSUM") as ps0:
                # W1cat = [W1 | W1@ablk_src | W1@ablk_dst] bf16
                w1cat = cpool.tile([P, 136], bf16)
                nc.vector.tensor_copy(out=w1cat[:, 0:128], in_=W1_sb)
                ps_w = ps0.tile([P, 8], f32, tag="pw")
                nc.tensor.matmul(out=ps_w[:], lhsT=W1T_sb, rhs=ablk_sb,
                                 start=True, stop=True)
                nc.vector.tensor_copy(out=w1cat[:, 128:136], in_=ps_w[:])

                # vsd = W2 @ [att_src2.T | att_dst2.T] -> [128, 2] bf16
                vsd = cpool.tile([P, 2], bf16)
                ps_v = ps0.tile([P, 2], f32, tag="pw")
                nc.tensor.matmul(out=ps_v[:], lhsT=W2T_sb, rhs=att2T_sb,
                                 start=True, stop=True)
                nc.vector.tensor_copy(out=vsd[:], in_=ps_v[:])

                # bias broadcast rows -> [128, *] f32
                b1bc = cpool.tile([P, 128], f32)
                ps_b = ps0.tile([P, 128], f32, tag="pw")
                nc.tensor.matmul(out=ps_b[:], lhsT=ones_f, rhs=b1_row,
                                 start=True, stop=True)
                nc.vector.tensor_copy(out=b1bc[:], in_=ps_b[:])
                b2bc = cpool.tile([P, 16], f32)
                ps_b2 = ps0.tile([P, 16], f32, tag="pw")
                nc.tensor.matmul(out=ps_b2[:], lhsT=ones_f, rhs=b2_row,
                                 start=True, stop=True)
                nc.vector.tensor_copy(out=b2bc[:], in_=ps_b2[:])

                # pad rows
                nc.sync.dma_start(out=tab1[N:N + 1, :], in_=pad1_row)
                # only row NPC is ever gathered (the pad target); rows
                # NPC+1..PADN are never referenced
                nc.sync.dma_start(out=ltab2[NPC:NPC + 1, :], in_=pad2_row)

                # ---- phase 1: full h1/a_src1 table; a_dst1 for own nodes ----
                with nc.named_scope("phase1"), \
                     tc.tile_pool(name="p1", bufs=4) as p1:
                    XC = 16
                    for c0 in range(0, NT1, XC):
                        nct = min(XC, NT1 - c0)
                        xc = p1.tile([P, XC * P], bf16, tag="xc")
                        nc.sync.dma_start(out=xc[:, 0:nct * P],
                                          in_=x_t[:, c0 * P:(c0 + nct) * P])
                        for k in range(nct):
                            ph = ps0.tile([P, 136], f32, tag="ph")
                            nc.tensor.matmul(out=ph[:],
                                             lhsT=xc[:, k * P:(k + 1) * P],
                                             rhs=w1cat[:], start=True, stop=True)
                            pk = p1.tile([P, ROW1], f32, tag="pk")
                            nc.vector.tensor_copy(out=pk[:, 0:64].bitcast(bf16),
                                                  in_=ph[:, 0:128])
                            nc.vector.tensor_copy(out=pk[:, 64:68],
                                                  in_=ph[:, 128:132])
                            nt = c0 + k
                            nr = min(P, N - nt * P)
                            nc.sync.dma_start(out=tab1[nt * P:nt * P + nr, :],
                                              in_=pk[0:nr, :])
                    for b in range(NB):
                        pa = ps0.tile([P, 4], f32, tag="pa")
                        nc.tensor.matmul(out=pa[:],
                                         lhsT=xo[:, b * P:(b + 1) * P],
                                         rhs=w1cat[:, 132:136],
                                         start=True, stop=True)
                        nc.vector.tensor_copy(out=adst1[:, b * 4:(b + 1) * 4],
                                              in_=pa[:])

            # ---- edge phases ----
            sbp = ctx.enter_context(tc.tile_pool(name="sbp", bufs=4))
            gpo = ctx.enter_context(tc.tile_pool(name="gpo", bufs=8))
            drp = ctx.enter_context(tc.tile_pool(name="drp", bufs=2))
            psE = ctx.enter_context(tc.tile_pool(name="psE", bufs=2, space="PSUM"))
            psF = ctx.enter_context(tc.tile_pool(name="psF", bufs=2, space="PSUM"))

            def edge_phase(layer):
                tabsrc = tab1 if layer == 1 else tab2
                srcsb = srcT1_sb if layer == 1 else srcT2_sb
                rowlen = ROW1 if layer == 1 else ROW2
                nh = HEADS if layer == 1 else 1
                mcols = 132 if layer == 1 else 129
                adst = adst1 if layer == 1 else adst2
                drow_t = None
                pb = None
                for g0 in range(0, TT, GRP):
                    ng = min(GRP, TT - g0)
                    if g0 % CHT == 0:
                        drow_t = drp.tile([1, CHT * P], bf16, tag="drow")
                        nc.sync.dma_start(out=drow_t[:],
                                          in_=dstrow[:, g0 * P:(g0 + CHT) * P])
                    off = (g0 % CHT) * P
                    pbc = psE.tile([P, GRP * P], f32, tag="pbc")
                    nc.tensor.matmul(out=pbc[:, 0:ng * P], lhsT=ones_bf,
                                     rhs=drow_t[:, off:off + ng * P],
                                     start=True, stop=True)
                    oht4 = sbp.tile([P, GRP * P], bf16, tag="oht4")
                    nc.vector.tensor_tensor(
                        out=oht4[:, 0:ng * P],
                        in0=iota_col.to_broadcast([P, ng * P]),
                        in1=pbc[:, 0:ng * P], op=AL.is_equal)
                    oh4 = sbp.tile([P, GRP * P], bf16, tag="oh4")
                    nc.vector.tensor_tensor(
                        out=oh4[:, 0:ng * P].rearrange("p (g j) -> p g j", g=ng),
                        in0=dstloc_bf[:, g0:g0 + ng]
                            .rearrange("p (g o) -> p g o", o=1)
                            .to_broadcast([P, ng, P]),
                        in1=iota_row_bf.rearrange("p (o j) -> p o j", o=1)
                            .to_broadcast([P, ng, P]),
                        op=AL.is_equal)
                    # gathers for the whole group into one buffer
                    g4 = gpo.tile([P, GRP * rowlen], f32, tag="g4")
                    for j in range(ng):
                        nc.gpsimd.indirect_dma_start(
                            out=g4[:, j * rowlen:(j + 1) * rowlen],
                            out_offset=None, in_=tabsrc[:],
                            in_offset=bass.IndirectOffsetOnAxis(
                                ap=srcsb[:, g0 + j:g0 + j + 1], axis=0))
                    # a_dst per edge via one-hot matmuls (bf16)
                    pD16 = psE.tile([P, GRP * 4], f32, tag="pD")
                    for j in range(ng):
                        bj = int(blk_of_tile[g0 + j])
                        nc.tensor.matmul(out=pD16[:, j * 4:j * 4 + nh],
                                         lhsT=oht4[:, j * P:(j + 1) * P],
                                         rhs=adst[:, bj * nh:(bj + 1) * nh],
                                         start=True, stop=True)
                    # e = a_src + a_dst, leaky, exp -- batched over the group
                    e16 = gpo.tile([P, GRP * 4], f32, tag="e16")
                    nc.vector.tensor_tensor(
                        out=e16[:, 0:ng * 4].rearrange("p (g h) -> p g h", g=ng)[:, :, 0:nh],
                        in0=g4[:, 0:ng * rowlen]
                            .rearrange("p (g r) -> p g r", g=ng)[:, :, 64:64 + nh],
                        in1=pD16[:, 0:ng * 4]
                            .rearrange("p (g h) -> p g h", g=ng)[:, :, 0:nh],
                        op=AL.add)
                    t16 = gpo.tile([P, GRP * 4], f32, tag="t16")
                    nc.vector.tensor_scalar_mul(out=t16[:, 0:ng * 4],
                                                in0=e16[:, 0:ng * 4], scalar1=NEG)
                    l16 = gpo.tile([P, GRP * 4], f32, tag="l16")
                    nc.vector.tensor_tensor(out=l16[:, 0:ng * 4],
                                            in0=e16[:, 0:ng * 4],
                                            in1=t16[:, 0:ng * 4], op=AL.max)
                    m4 = gpo.tile([P, GRP * 132], bf16, tag="m4")
                    nc.scalar.activation(
                        out=m4[:, 0:ng * 132]
                            .rearrange("p (g r) -> p g r", g=ng)[:, :, 128:132],
                        in_=l16[:, 0:ng * 4].rearrange("p (g h) -> p g h", g=ng),
                        func=AF.Exp)
                    g4b = g4[:, 0:ng * rowlen].bitcast(bf16)   # [P, ng*rowlen*2]
                    for j in range(ng):
                        nc.vector.tensor_tensor(
                            out=m4[:, j * 132:j * 132 + 128]
                                .rearrange("p (h c) -> p h c", h=nh),
                            in0=g4b[:, j * rowlen * 2:j * rowlen * 2 + 128]
                                .rearrange("p (h c) -> p h c", h=nh),
                            in1=m4[:, j * 132 + 128:j * 132 + 128 + nh]
                                .rearrange("p (h o) -> p h o", o=1)
                                .to_broadcast([P, nh, 128 // nh]),
                            op=AL.mult)
                    for j in range(ng):
                        gt = g0 + j
                        b = int(blk_of_tile[gt])
                        first = gt == tsl[b]
                        last = gt == tsl[b + 1] - 1
                        if first:
                            pb = psF.tile([P, mcols], f32, tag="pb")
                        nc.tensor.matmul(out=pb[:],
                                         lhsT=oh4[:, j * P:(j + 1) * P],
                                         rhs=m4[:, j * 132:j * 132 + mcols],
                                         start=first, stop=last)
                        if last:
                            finalize(layer, b, pb)

            def finalize(layer, b, pb):
                rows = min(P, NPC - b * P)
                nh = HEADS if layer == 1 else 1
                den = gpo.tile([P, 4], f32, tag="den")
                nc.vector.tensor_scalar_add(out=den[:, 0:nh],
                                            in0=pb[:, 128:128 + nh],
                                            scalar1=1e-16)
                rec = gpo.tile([P, 4], f32, tag="rec")
                nc.vector.reciprocal(out=rec[:, 0:nh], in_=den[:, 0:nh])
                if layer == 1:
                    tmp = sbp.tile([P, 128], f32, tag="tmp")
                    nc.vector.tensor_tensor(
                        out=tmp[:].rearrange("p (h c) -> p h c", h=nh),
                        in0=pb[:, 0:128].rearrange("p (h c) -> p h c", h=nh),
                        in1=rec[:, 0:nh].rearrange("p (h o) -> p h o", o=1)
                            .to_broadcast([P, nh, 128 // nh]),
                        op=AL.mult)
                    nc.vector.tensor_tensor(out=tmp[:], in0=tmp[:], in1=b1bc[:],
                                            op=AL.add)
                    nc.vector.tensor_scalar_max(out=tmp[:], in0=tmp[:], scalar1=0.0)
                    upk = sbp.tile([P, ROW1], f32, tag="upk")  # ROW1>=ROW2
                    nc.vector.tensor_copy(out=upk[:, 0:64].bitcast(bf16), in_=tmp[:])
                    pt = psF.tile([P, P], f32, tag="pt")
                    nc.tensor.transpose(out=pt[:], in_=tmp[:], identity=ident[:])
                    uT = sbp.tile([P, P], bf16, tag="uT")
                    nc.vector.tensor_copy(out=uT[:], in_=pt[:])
                    pa2 = psE.tile([P, 2], f32, tag="pD")
                    nc.tensor.matmul(out=pa2[:], lhsT=uT[:], rhs=vsd[:],
                                     start=True, stop=True)
                    nc.vector.tensor_copy(out=upk[:, 64:65], in_=pa2[:, 0:1])
                    nc.vector.tensor_copy(out=adst2[:, b:b + 1], in_=pa2[:, 1:2])
                    nc.sync.dma_start(out=ltab2[b * P:b * P + rows, :],
                                      in_=upk[0:rows, 0:ROW2])
                    if b % 7 == 6:
                        k = b // 7
                        nc.gpsimd.collective_compute(
                            "AllGather", mybir.AluOpType.bypass,
                            replica_groups=[list(range(C))],
                            ins=[ltab2[k * 896:(k + 1) * 896, :]],
                            outs=[tab2[k * 7168:(k + 1) * 7168, :]])
                else:
                    agg = sbp.tile([P, P], f32, tag="tmp")
                    nc.vector.tensor_scalar_mul(out=agg[:], in0=pb[:, 0:128],
                                                scalar1=rec[:, 0:1])
                    pt = psF.tile([P, P], f32, tag="pt")
                    nc.tensor.transpose(out=pt[:], in_=agg[:], identity=ident[:])
                    aT = sbp.tile([P, P], bf16, tag="uT")
                    nc.vector.tensor_copy(out=aT[:], in_=pt[:])
                    pz = psE.tile([P, 16], f32, tag="pD")
                    nc.tensor.matmul(out=pz[:], lhsT=aT[:], rhs=W2_sb,
                                     start=True, stop=True)
                    z = gpo.tile([P, 16], f32, tag="z")
                    nc.vector.tensor_tensor(out=z[:], in0=pz[:], in1=b2bc[:],
                                            op=AL.add)
                    mx = gpo.tile([P, 1], f32, tag="mx")
                    nc.vector.tensor_reduce(out=mx[:], in_=z[:],
                                            axis=mybir.AxisListType.X,
                                            op=AL.max, negate=True)  # -max
                    es = gpo.tile([P, 16], f32, tag="es")
                    ssum = gpo.tile([P, 1], f32, tag="ssum")
                    nc.scalar.activation(out=es[:], in_=z[:], func=AF.Exp,
                                         bias=mx[:], accum_out=ssum[:])
                    ls = gpo.tile([P, 1], f32, tag="ls")
                    nc.scalar.activation(out=ls[:], in_=ssum[:], func=AF.Ln)
                    sh = gpo.tile([P, 1], f32, tag="sh")
                    nc.vector.tensor_tensor(out=sh[:], in0=ls[:], in1=mx[:],
                                            op=AL.subtract)  # ln(s) - (-max)... see note
                    res = gpo.tile([P, 16], f32, tag="res")
                    nc.vector.tensor_scalar_sub(out=res[:], in0=z[:],
                                                scalar1=sh[:, 0:1])
                    nc.sync.dma_start(out=out_d[b * P:b * P + rows, :],
                                      in_=res[0:rows, :])

            with nc.named_scope("edge1"):
                edge_phase(1)

            with nc.named_scope("edge2"):
                edge_phase(2)

    from wait_fix import split_excess_waits
    split_excess_waits(nc)
    return nc


# log_softmax shift note: out = z - max - ln(sum(exp(z - max))).
# mx holds -max (negate=True). es = exp(z + mx), ssum = sum(es), ls = ln(ssum).
# shift = max + ls = ls - mx. res = z - shift.


def _host_arrays(x, W1, att_src1, att_dst1, b1, W2, att_src2, att_dst2, b2,
                 srcT1, srcT2, dstloc, TT):
    TTg = ((TT + CHT - 1) // CHT) * CHT
    xT = np.zeros((P, XPAD), bfnp)
    xT[:, 0:N] = np.asarray(x, np.float32).T.astype(bfnp)

    x_own = np.zeros((C, P, PADN), bfnp)
    for c in range(C):
        end = min(c * NPC + PADN, XPAD)
        x_own[c, :, 0:end - c * NPC] = xT[:, c * NPC:end]

    iota_row = np.broadcast_to(np.arange(P, dtype=np.int32)[None, :], (P, P))
    iota_col = np.arange(P, dtype=np.int32).reshape(P, 1)
    idxs = np.zeros((C, P, 3 * TT + P + 1), np.int32)
    for c in range(C):
        idxs[c, :, 0:TT] = srcT1[c].T
        idxs[c, :, TT:2 * TT] = srcT2[c].T
        idxs[c, :, 2 * TT:3 * TT] = dstloc[c].T
        idxs[c, :, 3 * TT:3 * TT + P] = iota_row
        idxs[c, :, 3 * TT + P:] = iota_col

    dstrow = np.zeros((C, 1, TTg * P), bfnp)
    for c in range(C):
        dstrow[c, 0, 0:TT * P] = dstloc[c].reshape(-1).astype(bfnp)

    NCBF = 128 + 128 + 8 + 16 + 128 + 2 + 128 + TT + 128
    cbf = np.zeros((C, P, NCBF), bfnp)
    W1f = np.asarray(W1, np.float32)
    cbf[:, :, 0:128] = W1f.astype(bfnp)
    cbf[:, :, 128:256] = W1f.T.astype(bfnp)
    ablk = np.zeros((128, 8), np.float32)
    for h in range(HEADS):
        ablk[h * HID:(h + 1) * HID, h] = np.asarray(att_src1, np.float32)[h]
        ablk[h * HID:(h + 1) * HID, 4 + h] = np.asarray(att_dst1, np.float32)[h]
    cbf[:, :, 256:264] = ablk.astype(bfnp)
    W2f = np.asarray(W2, np.float32)
    cbf[:, :, 264:280] = W2f.astype(bfnp)
    cbf[:, 0:16, 280:408] = W2f.T.astype(bfnp)
    cbf[:, 0:16, 408:409] = np.asarray(att_src2, np.float32).T.astype(bfnp)
    cbf[:, 0:16, 409:410] = np.asarray(att_dst2, np.float32).T.astype(bfnp)
    cbf[:, 0:1, 410:538] = np.ones((1, 128), bfnp)
    for c in range(C):
        cbf[c, :, 538:538 + TT] = np.ascontiguousarray(dstloc[c].T).astype(bfnp)
    cbf[:, :, 538 + TT:538 + TT + 128] = np.broadcast_to(
        np.arange(128, dtype=np.float32)[None, None, :], (C, P, 128)).astype(bfnp)

    NCF = 128 + 16 + ROW1 + ROW2 + 128
    cf = np.zeros((1, NCF), np.float32)
    cf[0, 0:128] = np.asarray(b1, np.float32)
    cf[0, 128:144] = np.asarray(b2, np.float32)
    pad1 = np.zeros(ROW1, np.float32)
    pad1[64:68] = -1e9
    cf[0, 144:144 + ROW1] = pad1
    pad2 = np.zeros(ROW2, np.float32)
    pad2[64] = -1e9
    cf[0, 144 + ROW1:144 + ROW1 + ROW2] = pad2
    cf[0, 144 + ROW1 + ROW2:NCF] = 1.0

    return xT, x_own, idxs, dstrow, cbf, cf


_CACHE = {}


def kernel(x, edge_index, W1, att_src1, att_dst1, b1, W2, att_src2, att_dst2, b2,
           _trace=False, _tmpdir=None):
    srcT1, srcT2, dstloc, TB, TT, tile_starts = _prep(np.asarray(edge_index))
    xT, x_own, idxs, dstrow, cbf, cf = _host_arrays(
        x, W1, att_src1, att_dst1, b1, W2, att_src2, att_dst2, b2,
        srcT1, srcT2, dstloc, TT)

    key = (TT, tuple(int(t) for t in TB))
    if key not in _CACHE:
        _CACHE[key] = _build(TT, TB, tile_starts)
    nc = _CACHE[key]

    in_maps = []
    for c in range(C):
        in_maps.append({
            "x_t": xT, "x_own": np.ascontiguousarray(x_own[c]),
            "idxs": np.ascontiguousarray(idxs[c]),
            "dstrow": np.ascontiguousarray(dstrow[c]),
            "cbf": np.ascontiguousarray(cbf[c]), "cf": cf,
        })

    res = run_bass_kernel_spmd(nc, in_maps, list(range(C)), trace=_trace,
                               tmpdir=_tmpdir)
    out = np.concatenate([res.results[c]["out"] for c in range(C)], axis=0)
    kernel.last_results = res
    return out.astype(np.float32)


# revision 12
# speedup vs baseline: 1.6113x; 1.1003x over previous
"""GAT (2-layer, PyG-style) Trainium2 kernel, edge-parallel across 8 NeuronCores.

Self-contained: host-side numpy preprocessing (sharding / edge sorting / index
tables), Bass/Tile kernel build, SPMD execution on cores 0-7, gather of the
full [50000, 16] log-softmax output.

Strategy:
  - edges (plus self loops) sorted by dst; dst-range sharded: core d owns dst in
    [d*6250, (d+1)*6250) so all segment reductions are core-local.
  - per 128-node block, edges are tiled into 128-edge tiles; segment softmax
    sums are one-hot matmuls accumulated in PSUM (no scatter).
  - softmax without max subtraction (logits are O(1) for this model; verified
    offline: max |e| < 4, so exp never overflows; self loops keep denom > 0).
  - layer-2 messages: W2 is linear and applied after aggregation (heads=1), so
    aggregate relu(h1) with layer-2 attention weights first, then matmul W2.
  - node tables replicated: each core computes the full [N] layer-1 table; the
    layer-2 table is AllGathered (each core produces its own 6250 rows).
"""
import numpy as np
import ml_dtypes

import concourse.bass as bass
import concourse.mybir as mybir
import concourse.tile as tile
from concourse.bass_utils import run_bass_kernel_spmd

bfnp = ml_dtypes.bfloat16

N = 50000
E = 800000
IN_CH = 128
HID = 32
HEADS = 4
OUT_CH = 16
NEG = 0.2
C = 8
NPC = N // C               # 6250 nodes per core
P = 128
NB = (NPC + P - 1) // P    # 49 blocks per core
PADN = NB * P              # 6272 rows per core stripe in allgathered table
ROW1 = 68                  # tab1 row: 64 f32 words (128 bf16 h) + 4 f32 a_src
ROW2 = 65                  # tab2 row: 64 f32 words (128 bf16 u) + 1 f32 a_src2
GRP = 4                    # one-hot generation group (one 512-wide psum bank)
CHT = 64                   # dstrow chunk length in tiles
NT1 = (N + P - 1) // P     # 391 node tiles (50048 padded)
XPAD = NT1 * P

f32 = mybir.dt.float32
bf16 = mybir.dt.bfloat16
i32 = mybir.dt.int32


def _split_excess_waits(nc, keep=1):
    """TRN2 walrus codegen rejects instructions carrying more than one
    sync-wait command; hoist extras onto same-engine NoOp carriers."""
    _skip = (mybir.InstEventSemaphore,)
    n_split = 0
    for fn in nc.m.functions:
        for bb in fn.blocks:
            newlist = []
            for ins_ in bb.instructions:
                si = ins_.sync_info
                if (si is not None and si.on_wait and len(si.on_wait) > keep
                        and not isinstance(ins_, _skip)):
                    waits = list(si.on_wait)
                    extra, rest = waits[:-keep], waits[-keep:]
                    for j, w in enumerate(extra):
                        newlist.append(mybir.InstNoOp(
                            name=f"{ins_.name}-wn{j}", engine=ins_.engine,
                            ins=[], outs=[],
                            sync_info=mybir.SyncInfo(on_wait=[w], on_update=[])))
                    ins_.sync_info = mybir.SyncInfo(
                        on_wait=rest, on_update=list(si.on_update))
                    n_split += 1
                newlist.append(ins_)
            bb.instructions[:] = newlist
    return n_split


def _prep(edge_index):
    """Sort/shard/tile edges. Returns per-core index arrays + block tile counts."""
    src = np.concatenate([np.asarray(edge_index[0]), np.arange(N, dtype=np.int64)])
    dst = np.concatenate([np.asarray(edge_index[1]), np.arange(N, dtype=np.int64)])
    order = np.argsort(dst, kind="stable")
    src = src[order].astype(np.int64)
    dst = dst[order].astype(np.int64)

    core_of = dst // NPC
    per_core = []
    counts = np.zeros((C, NB), np.int64)
    for c in range(C):
        m = core_of == c
        s, d = src[m], dst[m]
        loc = d - c * NPC
        blk = loc // P
        np.add.at(counts[c], blk, 1)
        per_core.append((s, loc, blk))

    TB = np.maximum(1, (counts.max(axis=0) + P - 1) // P)
    TT = int(TB.sum())
    tile_starts = np.zeros(NB + 1, np.int64)
    tile_starts[1:] = np.cumsum(TB)

    srcT1 = np.zeros((C, TT, P), np.int32)
    srcT2 = np.zeros((C, TT, P), np.int32)
    dstloc = np.zeros((C, TT, P), np.int32)

    for c in range(C):
        s, loc, blk = per_core[c]
        for b in range(NB):
            m = blk == b
            sb_, lb = s[m], loc[m] - b * P
            n = len(sb_)
            cap = int(TB[b]) * P
            own = sb_ // NPC
            ln = sb_ - own * NPC
            s1 = np.full(cap, NPC, np.int64)        # core0 stripe pad (a_src=-1e9)
            s1[:n] = own * PADN + ln
            # chunked allgather layout: chunk k = ln//896 holds 8 stripes of 896
            s2 = np.full(cap, 6 * 7168 + 874, np.int64)   # core0 pad row (ln=6250)
            s2[:n] = (ln // 896) * 7168 + own * 896 + (ln % 896)
            dl = np.zeros(cap, np.int64)            # pad edges hit col 0, ex=0
            dl[:n] = lb
            t0 = tile_starts[b]
            srcT1[c, t0:t0 + TB[b]] = s1.reshape(int(TB[b]), P)
            srcT2[c, t0:t0 + TB[b]] = s2.reshape(int(TB[b]), P)
            dstloc[c, t0:t0 + TB[b]] = dl.reshape(int(TB[b]), P)

    return srcT1, srcT2, dstloc, TB, TT, tile_starts


def _build(TT, TB, tile_starts):
    """Build the SPMD Bass graph (identical for all cores)."""
    TTg = ((TT + CHT - 1) // CHT) * CHT
    NCBF = 128 + 128 + 8 + 16 + 128 + 2 + 128 + TT + 128
    NCF = 128 + 16 + ROW1 + ROW2 + 128
    tsl = [int(t) for t in tile_starts]
    blk_of_tile = np.zeros(TT, np.int64)
    for b in range(NB):
        blk_of_tile[tsl[b]:tsl[b + 1]] = b

    nc = bass.Bass()
    x_own = nc.declare_dram_parameter("x_own", [P, PADN], bf16, isOutput=False)
    idxs = nc.declare_dram_parameter("idxs", [P, 3 * TT + P + 1], i32, isOutput=False)
    dstrow = nc.declare_dram_parameter("dstrow", [1, TTg * P], bf16, isOutput=False)
    cbf = nc.declare_dram_parameter("cbf", [P, NCBF], bf16, isOutput=False)
    cf = nc.declare_dram_parameter("cf", [1, NCF], f32, isOutput=False)
    out_d = nc.declare_dram_parameter("out", [NPC, OUT_CH], f32, isOutput=True)

    ltab1 = nc.dram_tensor("ltab1", [PADN, ROW1], f32)
    tab1 = nc.dram_tensor("tab1", [C * PADN, ROW1], f32, addr_space="Shared")
    ltab2 = nc.dram_tensor("ltab2", [PADN, ROW2], f32)
    tab2 = nc.dram_tensor("tab2", [C * PADN, ROW2], f32, addr_space="Shared")

    AL = mybir.AluOpType
    AF = mybir.ActivationFunctionType

    with tile.TileContext(nc) as tc:
        from contextlib import ExitStack
        with ExitStack() as ctx:
            cpool = ctx.enter_context(tc.tile_pool(name="const", bufs=1))

            # ---- constants ----
            ci = cpool.tile([P, 3 * TT + P + 1], i32)
            nc.sync.dma_start(out=ci[:], in_=idxs[:])
            srcT1_sb = ci[:, 0:TT]
            srcT2_sb = ci[:, TT:2 * TT]
            dstlocT_sb = ci[:, 2 * TT:3 * TT]
            iota_row = ci[:, 3 * TT:3 * TT + P]
            iota_col = ci[:, 3 * TT + P:3 * TT + P + 1]

            cb = cpool.tile([P, NCBF], bf16)
            nc.sync.dma_start(out=cb[:], in_=cbf[:])
            W1_sb = cb[:, 0:128]
            W1T_sb = cb[:, 128:256]
            ablk_sb = cb[:, 256:264]
            W2_sb = cb[:, 264:280]
            W2T_sb = cb[0:16, 280:408]
            att2T_sb = cb[0:16, 408:410]
            ones_bf = cb[0:1, 410:538]  # ones row [1, 128]
            dstloc_bf = cb[:, 538:538 + TT]
            iota_row_bf = cb[:, 538 + TT:538 + TT + P]

            cfs = cpool.tile([1, NCF], f32)
            nc.sync.dma_start(out=cfs[:], in_=cf[:])
            b1_row = cfs[:, 0:128]
            b2_row = cfs[:, 128:144]
            pad1_row = cfs[:, 144:144 + ROW1]
            pad2_row = cfs[:, 144 + ROW1:144 + ROW1 + ROW2]
            ones_f = cfs[:, 144 + ROW1 + ROW2:NCF]  # ones row [1, 128]

            xo = cpool.tile([P, PADN], bf16)
            nc.sync.dma_start(out=xo[:], in_=x_own[:])

            # prime DVE on const DMAs (keeps later waits <= 1 per instruction)
            pr = cpool.tile([P, 1], f32)
            nc.vector.tensor_tensor(out=pr[:], in0=ci[:, 0:1], in1=cb[:, 0:1],
                                    op=AL.add)
            nc.vector.tensor_tensor(out=pr[0:1, :], in0=cfs[0:1, 0:1],
                                    in1=xo[0:1, 0:1], op=AL.add)

            adst1 = cpool.tile([P, NB * HEADS], bf16)
            adst2 = cpool.tile([P, NB], bf16)

            from concourse.masks import make_identity
            ident = cpool.tile([P, P], f32)
            make_identity(nc, ident[:])

            with tc.tile_pool(name="ps0", bufs=2, space="PSUM") as ps0:
                # W1cat = [W1 | W1@ablk_src | W1@ablk_dst] bf16
                w1cat = cpool.tile([P, 136], bf16)
                nc.vector.tensor_copy(out=w1cat[:, 0:128], in_=W1_sb)
                ps_w = ps0.tile([P, 8], f32, tag="pw")
                nc.tensor.matmul(out=ps_w[:], lhsT=W1T_sb, rhs=ablk_sb,
                                 start=True, stop=True)
                nc.vector.tensor_copy(out=w1cat[:, 128:136], in_=ps_w[:])

                # vsd = W2 @ [att_src2.T | att_dst2.T] -> [128, 2] bf16
                vsd = cpool.tile([P, 2], bf16)
                ps_v = ps0.tile([P, 2], f32, tag="pw")
                nc.tensor.matmul(out=ps_v[:], lhsT=W2T_sb, rhs=att2T_sb,
                                 start=True, stop=True)
                nc.vector.tensor_copy(out=vsd[:], in_=ps_v[:])

                # bias broadcast rows -> [128, *] f32
                b1bc = cpool.tile([P, 128], f32)
                ps_b = ps0.tile([P, 128], f32, tag="pw")
                nc.tensor.matmul(out=ps_b[:], lhsT=ones_f, rhs=b1_row,
                                 start=True, stop=True)
                nc.vector.tensor_copy(out=b1bc[:], in_=ps_b[:])
                b2bc = cpool.tile([P, 16], f32)
                ps_b2 = ps0.tile([P, 16], f32, tag="pw")
                nc.tensor.matmul(out=ps_b2[:], lhsT=ones_f, rhs=b2_row,
                                 start=True, stop=True)
                nc.vector.tensor_copy(out=b2bc[:], in_=ps_b2[:])

                # pad rows
                nc.sync.dma_start(out=ltab1[NPC:NPC + 1, :], in_=pad1_row)
                # only row NPC is ever gathered (the pad target); rows
                # NPC+1..PADN are never referenced
                nc.sync.dma_start(out=ltab2[NPC:NPC + 1, :], in_=pad2_row)

                # ---- phase 1: own 49 blocks of h1/a_src1 from x_own, then
                # AllGather the stripe table; a_dst1 for own nodes ----
                with nc.named_scope("phase1"), \
                     tc.tile_pool(name="p1", bufs=4) as p1:
                    for b in range(NB):
                        ph = ps0.tile([P, 136], f32, tag="ph")
                        nc.tensor.matmul(out=ph[:],
                                         lhsT=xo[:, b * P:(b + 1) * P],
                                         rhs=w1cat[:], start=True, stop=True)
                        pk = p1.tile([P, ROW1], f32, tag="pk")
                        nc.vector.tensor_copy(out=pk[:, 0:64].bitcast(bf16),
                                              in_=ph[:, 0:128])
                        nc.vector.tensor_copy(out=pk[:, 64:68],
                                              in_=ph[:, 128:132])
                        rows = min(P, NPC - b * P)
                        nc.sync.dma_start(out=ltab1[b * P:b * P + rows, :],
                                          in_=pk[0:rows, :])
                    nc.gpsimd.collective_compute(
                        "AllGather", mybir.AluOpType.bypass,
                        replica_groups=[list(range(C))],
                        ins=[ltab1[:]], outs=[tab1[:]])
                    for b in range(NB):
                        pa = ps0.tile([P, 4], f32, tag="pa")
                        nc.tensor.matmul(out=pa[:],
                                         lhsT=xo[:, b * P:(b + 1) * P],
                                         rhs=w1cat[:, 132:136],
                                         start=True, stop=True)
                        nc.vector.tensor_copy(out=adst1[:, b * 4:(b + 1) * 4],
                                              in_=pa[:])

            # ---- edge phases ----
            sbp = ctx.enter_context(tc.tile_pool(name="sbp", bufs=4))
            gpo = ctx.enter_context(tc.tile_pool(name="gpo", bufs=8))
            drp = ctx.enter_context(tc.tile_pool(name="drp", bufs=2))
            psE = ctx.enter_context(tc.tile_pool(name="psE", bufs=2, space="PSUM"))
            psF = ctx.enter_context(tc.tile_pool(name="psF", bufs=2, space="PSUM"))

            def edge_phase(layer):
                tabsrc = tab1 if layer == 1 else tab2
                srcsb = srcT1_sb if layer == 1 else srcT2_sb
                rowlen = ROW1 if layer == 1 else ROW2
                nh = HEADS if layer == 1 else 1
                mcols = 132 if layer == 1 else 129
                adst = adst1 if layer == 1 else adst2
                drow_t = None
                pb = None
                for g0 in range(0, TT, GRP):
                    ng = min(GRP, TT - g0)
                    if g0 % CHT == 0:
                        drow_t = drp.tile([1, CHT * P], bf16, tag="drow")
                        nc.sync.dma_start(out=drow_t[:],
                                          in_=dstrow[:, g0 * P:(g0 + CHT) * P])
                    off = (g0 % CHT) * P
                    pbc = psE.tile([P, GRP * P], f32, tag="pbc")
                    nc.tensor.matmul(out=pbc[:, 0:ng * P], lhsT=ones_bf,
                                     rhs=drow_t[:, off:off + ng * P],
                                     start=True, stop=True)
                    oht4 = sbp.tile([P, GRP * P], bf16, tag="oht4")
                    nc.vector.tensor_tensor(
                        out=oht4[:, 0:ng * P],
                        in0=iota_col.to_broadcast([P, ng * P]),
                        in1=pbc[:, 0:ng * P], op=AL.is_equal)
                    oh4 = sbp.tile([P, GRP * P], bf16, tag="oh4")
                    nc.vector.tensor_tensor(
                        out=oh4[:, 0:ng * P].rearrange("p (g j) -> p g j", g=ng),
                        in0=dstloc_bf[:, g0:g0 + ng]
                            .rearrange("p (g o) -> p g o", o=1)
                            .to_broadcast([P, ng, P]),
                        in1=iota_row_bf.rearrange("p (o j) -> p o j", o=1)
                            .to_broadcast([P, ng, P]),
                        op=AL.is_equal)
                    # gathers for the whole group into one buffer
                    g4 = gpo.tile([P, GRP * rowlen], f32, tag="g4")
                    for j in range(ng):
                        nc.gpsimd.indirect_dma_start(
                            out=g4[:, j * rowlen:(j + 1) * rowlen],
                            out_offset=None, in_=tabsrc[:],
                            in_offset=bass.IndirectOffsetOnAxis(
                                ap=srcsb[:, g0 + j:g0 + j + 1], axis=0))
                    # a_dst per edge via one-hot matmuls (bf16)
                    pD16 = psE.tile([P, GRP * 4], f32, tag="pD")
                    for j in range(ng):
                        bj = int(blk_of_tile[g0 + j])
                        nc.tensor.matmul(out=pD16[:, j * 4:j * 4 + nh],
                                         lhsT=oht4[:, j * P:(j + 1) * P],
                                         rhs=adst[:, bj * nh:(bj + 1) * nh],
                                         start=True, stop=True)
                    # e = a_src + a_dst, leaky, exp -- batched over the group
                    e16 = gpo.tile([P, GRP * 4], f32, tag="e16")
                    nc.vector.tensor_tensor(
                        out=e16[:, 0:ng * 4].rearrange("p (g h) -> p g h", g=ng)[:, :, 0:nh],
                        in0=g4[:, 0:ng * rowlen]
                            .rearrange("p (g r) -> p g r", g=ng)[:, :, 64:64 + nh],
                        in1=pD16[:, 0:ng * 4]
                            .rearrange("p (g h) -> p g h", g=ng)[:, :, 0:nh],
                        op=AL.add)
                    t16 = gpo.tile([P, GRP * 4], f32, tag="t16")
                    nc.vector.tensor_scalar_mul(out=t16[:, 0:ng * 4],
                                                in0=e16[:, 0:ng * 4], scalar1=NEG)
                    l16 = gpo.tile([P, GRP * 4], f32, tag="l16")
                    nc.vector.tensor_tensor(out=l16[:, 0:ng * 4],
                                            in0=e16[:, 0:ng * 4],
                                            in1=t16[:, 0:ng * 4], op=AL.max)
                    m4 = gpo.tile([P, GRP * 132], bf16, tag="m4")
                    nc.scalar.activation(
                        out=m4[:, 0:ng * 132]
                            .rearrange("p (g r) -> p g r", g=ng)[:, :, 128:132],
                        in_=l16[:, 0:ng * 4].rearrange("p (g h) -> p g h", g=ng),
                        func=AF.Exp)
                    g4b = g4[:, 0:ng * rowlen].bitcast(bf16)   # [P, ng*rowlen*2]
                    for j in range(ng):
                        nc.vector.tensor_tensor(
                            out=m4[:, j * 132:j * 132 + 128]
                                .rearrange("p (h c) -> p h c", h=nh),
                            in0=g4b[:, j * rowlen * 2:j * rowlen * 2 + 128]
                                .rearrange("p (h c) -> p h c", h=nh),
                            in1=m4[:, j * 132 + 128:j * 132 + 128 + nh]
                                .rearrange("p (h o) -> p h o", o=1)
                                .to_broadcast([P, nh, 128 // nh]),
                            op=AL.mult)
                    for j in range(ng):
                        gt = g0 + j
                        b = int(blk_of_tile[gt])
                        first = gt == tsl[b]
                        last = gt == tsl[b + 1] - 1
                        if first:
                            pb = psF.tile([P, mcols], f32, tag="pb")
                        nc.tensor.matmul(out=pb[:],
                                         lhsT=oh4[:, j * P:(j + 1) * P],
                                         rhs=m4[:, j * 132:j * 132 + mcols],
                                         start=first, stop=last)
                        if last:
                            finalize(layer, b, pb)

            def finalize(layer, b, pb):
                rows = min(P, NPC - b * P)
                nh = HEADS if layer == 1 else 1
                den = gpo.tile([P, 4], f32, tag="den")
                nc.vector.tensor_scalar_add(out=den[:, 0:nh],
                                            in0=pb[:, 128:128 + nh],
                                            scalar1=1e-16)
                rec = gpo.tile([P, 4], f32, tag="rec")
                nc.vector.reciprocal(out=rec[:, 0:nh], in_=den[:, 0:nh])
                if layer == 1:
                    tmp = sbp.tile([P, 128], f32, tag="tmp")
                    nc.vector.tensor_tensor(
                        out=tmp[:].rearrange("p (h c) -> p h c", h=nh),
                        in0=pb[:, 0:128].rearrange("p (h c) -> p h c", h=nh),
                        in1=rec[:, 0:nh].rearrange("p (h o) -> p h o", o=1)
                            .to_broadcast([P, nh, 128 // nh]),
                        op=AL.mult)
                    nc.vector.tensor_tensor(out=tmp[:], in0=tmp[:], in1=b1bc[:],
                                            op=AL.add)
                    nc.vector.tensor_scalar_max(out=tmp[:], in0=tmp[:], scalar1=0.0)
                    upk = sbp.tile([P, ROW1], f32, tag="upk")  # ROW1>=ROW2
                    nc.vector.tensor_copy(out=upk[:, 0:64].bitcast(bf16), in_=tmp[:])
                    pt = psF.tile([P, P], f32, tag="pt")
                    nc.tensor.transpose(out=pt[:], in_=tmp[:], identity=ident[:])
                    uT = sbp.tile([P, P], bf16, tag="uT")
                    nc.vector.tensor_copy(out=uT[:], in_=pt[:])
                    pa2 = psE.tile([P, 2], f32, tag="pD")
                    nc.tensor.matmul(out=pa2[:], lhsT=uT[:], rhs=vsd[:],
                                     start=True, stop=True)
                    nc.vector.tensor_copy(out=upk[:, 64:65], in_=pa2[:, 0:1])
                    nc.vector.tensor_copy(out=adst2[:, b:b + 1], in_=pa2[:, 1:2])
                    nc.sync.dma_start(out=ltab2[b * P:b * P + rows, :],
                                      in_=upk[0:rows, 0:ROW2])
                    if b % 7 == 6:
                        k = b // 7
                        nc.gpsimd.collective_compute(
                            "AllGather", mybir.AluOpType.bypass,
                            replica_groups=[list(range(C))],
                            ins=[ltab2[k * 896:(k + 1) * 896, :]],
                            outs=[tab2[k * 7168:(k + 1) * 7168, :]])
                else:
                    agg = sbp.tile([P, P], f32, tag="tmp")
                    nc.vector.tensor_scalar_mul(out=agg[:], in0=pb[:, 0:128],
                                                scalar1=rec[:, 0:1])
                    pt = psF.tile([P, P], f32, tag="pt")
                    nc.tensor.transpose(out=pt[:], in_=agg[:], identity=ident[:])
                    aT = sbp.tile([P, P], bf16, tag="uT")
                    nc.vector.tensor_copy(out=aT[:], in_=pt[:])
                    pz = psE.tile([P, 16], f32, tag="pD")
                    nc.tensor.matmul(out=pz[:], lhsT=aT[:], rhs=W2_sb,
                                     start=True, stop=True)
                    z = gpo.tile([P, 16], f32, tag="z")
                    nc.vector.tensor_tensor(out=z[:], in0=pz[:], in1=b2bc[:],
                                            op=AL.add)
                    mx = gpo.tile([P, 1], f32, tag="mx")
                    nc.vector.tensor_reduce(out=mx[:], in_=z[:],
                                            axis=mybir.AxisListType.X,
                                            op=AL.max, negate=True)  # -max
                    es = gpo.tile([P, 16], f32, tag="es")
                    ssum = gpo.tile([P, 1], f32, tag="ssum")
                    nc.scalar.activation(out=es[:], in_=z[:], func=AF.Exp,
                                         bias=mx[:], accum_out=ssum[:])
                    ls = gpo.tile([P, 1], f32, tag="ls")
                    nc.scalar.activation(out=ls[:], in_=ssum[:], func=AF.Ln)
                    sh = gpo.tile([P, 1], f32, tag="sh")
                    nc.vector.tensor_tensor(out=sh[:], in0=ls[:], in1=mx[:],
                                            op=AL.subtract)  # ln(s) - (-max)... see note
                    res = gpo.tile([P, 16], f32, tag="res")
                    nc.vector.tensor_scalar_sub(out=res[:], in0=z[:],
                                                scalar1=sh[:, 0:1])
                    nc.sync.dma_start(out=out_d[b * P:b * P + rows, :],
                                      in_=res[0:rows, :])

            with nc.named_scope("edge1"):
                edge_phase(1)

            with nc.named_scope("edge2"):
                edge_phase(2)

    _split_excess_waits(nc)
    return nc


# log_softmax shift note: out = z - max - ln(sum(exp(z - max))).
# mx holds -max (negate=True). es = exp(z + mx), ssum = sum(es), ls = ln(ssum).
# shift = max + ls = ls - mx. res = z - shift.


def _host_arrays(x, W1, att_src1, att_dst1, b1, W2, att_src2, att_dst2, b2,
                 srcT1, srcT2, dstloc, TT):
    TTg = ((TT + CHT - 1) // CHT) * CHT
    xT = np.zeros((P, XPAD), bfnp)
    xT[:, 0:N] = np.asarray(x, np.float32).T.astype(bfnp)

    x_own = np.zeros((C, P, PADN), bfnp)
    for c in range(C):
        end = min(c * NPC + PADN, XPAD)
        x_own[c, :, 0:end - c * NPC] = xT[:, c * NPC:end]

    iota_row = np.broadcast_to(np.arange(P, dtype=np.int32)[None, :], (P, P))
    iota_col = np.arange(P, dtype=np.int32).reshape(P, 1)
    idxs = np.zeros((C, P, 3 * TT + P + 1), np.int32)
    for c in range(C):
        idxs[c, :, 0:TT] = srcT1[c].T
        idxs[c, :, TT:2 * TT] = srcT2[c].T
        idxs[c, :, 2 * TT:3 * TT] = dstloc[c].T
        idxs[c, :, 3 * TT:3 * TT + P] = iota_row
        idxs[c, :, 3 * TT + P:] = iota_col

    dstrow = np.zeros((C, 1, TTg * P), bfnp)
    for c in range(C):
        dstrow[c, 0, 0:TT * P] = dstloc[c].reshape(-1).astype(bfnp)

    NCBF = 128 + 128 + 8 + 16 + 128 + 2 + 128 + TT + 128
    cbf = np.zeros((C, P, NCBF), bfnp)
    W1f = np.asarray(W1, np.float32)
    cbf[:, :, 0:128] = W1f.astype(bfnp)
    cbf[:, :, 128:256] = W1f.T.astype(bfnp)
    ablk = np.zeros((128, 8), np.float32)
    for h in range(HEADS):
        ablk[h * HID:(h + 1) * HID, h] = np.asarray(att_src1, np.float32)[h]
        ablk[h * HID:(h + 1) * HID, 4 + h] = np.asarray(att_dst1, np.float32)[h]
    cbf[:, :, 256:264] = ablk.astype(bfnp)
    W2f = np.asarray(W2, np.float32)
    cbf[:, :, 264:280] = W2f.astype(bfnp)
    cbf[:, 0:16, 280:408] = W2f.T.astype(bfnp)
    cbf[:, 0:16, 408:409] = np.asarray(att_src2, np.float32).T.astype(bfnp)
    cbf[:, 0:16, 409:410] = np.asarray(att_dst2, np.float32).T.astype(bfnp)
    cbf[:, 0:1, 410:538] = np.ones((1, 128), bfnp)
    for c in range(C):
        cbf[c, :, 538:538 + TT] = np.ascontiguousarray(dstloc[c].T).astype(bfnp)
    cbf[:, :, 538 + TT:538 + TT + 128] = np.broadcast_to(
        np.arange(128, dtype=np.float32)[None, None, :], (C, P, 128)).astype(bfnp)

    NCF = 128 + 16 + ROW1 + ROW2 + 128
    cf = np.zeros((1, NCF), np.float32)
    cf[0, 0:128] = np.asarray(b1, np.float32)
    cf[0, 128:144] = np.asarray(b2, np.float32)
    pad1 = np.zeros(ROW1, np.float32)
    pad1[64:68] = -1e9
    cf[0, 144:144 + ROW1] = pad1
    pad2 = np.zeros(ROW2, np.float32)
    pad2[64] = -1e9
    cf[0, 144 + ROW1:144 + ROW1 + ROW2] = pad2
    cf[0, 144 + ROW1 + ROW2:NCF] = 1.0

    return xT, x_own, idxs, dstrow, cbf, cf


_CACHE = {}


def kernel(x, edge_index, W1, att_src1, att_dst1, b1, W2, att_src2, att_dst2, b2,
           _trace=False, _tmpdir=None):
    srcT1, srcT2, dstloc, TB, TT, tile_starts = _prep(np.asarray(edge_index))
    xT, x_own, idxs, dstrow, cbf, cf = _host_arrays(
        x, W1, att_src1, att_dst1, b1, W2, att_src2, att_dst2, b2,
        srcT1, srcT2, dstloc, TT)

    key = (TT, tuple(int(t) for t in TB))
    if key not in _CACHE:
        _CACHE[key] = _build(TT, TB, tile_starts)
    nc = _CACHE[key]

    in_maps = []
    for c in range(C):
        in_maps.append({
            "x_own": np.ascontiguousarray(x_own[c]),
            "idxs": np.ascontiguousarray(idxs[c]),
            "dstrow": np.ascontiguousarray(dstrow[c]),
            "cbf": np.ascontiguousarray(cbf[c]), "cf": cf,
        })

    res = run_bass_kernel_spmd(nc, in_maps, list(range(C)), trace=_trace,
                               tmpdir=_tmpdir)
    out = np.concatenate([res.results[c]["out"] for c in range(C)], axis=0)
    kernel.last_results = res
    return out.astype(np.float32)


# revision 15
# speedup vs baseline: 1.6171x; 1.0036x over previous
"""GAT (2-layer, PyG-style) Trainium2 kernel, edge-parallel across 8 NeuronCores.

Self-contained: host-side numpy preprocessing (sharding / edge sorting / index
tables), Bass/Tile kernel build, SPMD execution on cores 0-7, gather of the
full [50000, 16] log-softmax output.

Strategy:
  - edges (plus self loops) sorted by dst; dst-range sharded: core d owns dst in
    [d*6250, (d+1)*6250) so all segment reductions are core-local.
  - per 128-node block, edges are tiled into 128-edge tiles; segment softmax
    sums are one-hot matmuls accumulated in PSUM (no scatter).
  - softmax without max subtraction (logits are O(1) for this model; verified
    offline: max |e| < 4, so exp never overflows; self loops keep denom > 0).
  - layer-2 messages: W2 is linear and applied after aggregation (heads=1), so
    aggregate relu(h1) with layer-2 attention weights first, then matmul W2.
  - node tables replicated: each core computes the full [N] layer-1 table; the
    layer-2 table is AllGathered (each core produces its own 6250 rows).
"""
import numpy as np
import ml_dtypes

import concourse.bass as bass
import concourse.mybir as mybir
import concourse.tile as tile
from concourse.bass_utils import run_bass_kernel_spmd

bfnp = ml_dtypes.bfloat16

N = 50000
E = 800000
IN_CH = 128
HID = 32
HEADS = 4
OUT_CH = 16
NEG = 0.2
C = 8
NPC = N // C               # 6250 nodes per core
P = 128
NB = (NPC + P - 1) // P    # 49 blocks per core
PADN = NB * P              # 6272 rows per core stripe in allgathered table
ROW1 = 68                  # tab1 row: 64 f32 words (128 bf16 h) + 4 f32 a_src
ROW2 = 65                  # tab2 row: 64 f32 words (128 bf16 u) + 1 f32 a_src2
GRP = 4                    # one-hot generation group (one 512-wide psum bank)
CHT = 64                   # dstrow chunk length in tiles
NT1 = (N + P - 1) // P     # 391 node tiles (50048 padded)
XPAD = NT1 * P

f32 = mybir.dt.float32
bf16 = mybir.dt.bfloat16
i32 = mybir.dt.int32


def _split_excess_waits(nc, keep=1):
    """TRN2 walrus codegen rejects instructions carrying more than one
    sync-wait command; hoist extras onto same-engine NoOp carriers."""
    _skip = (mybir.InstEventSemaphore,)
    n_split = 0
    for fn in nc.m.functions:
        for bb in fn.blocks:
            newlist = []
            for ins_ in bb.instructions:
                si = ins_.sync_info
                if (si is not None and si.on_wait and len(si.on_wait) > keep
                        and not isinstance(ins_, _skip)):
                    waits = list(si.on_wait)
                    extra, rest = waits[:-keep], waits[-keep:]
                    for j, w in enumerate(extra):
                        newlist.append(mybir.InstNoOp(
                            name=f"{ins_.name}-wn{j}", engine=ins_.engine,
                            ins=[], outs=[],
                            sync_info=mybir.SyncInfo(on_wait=[w], on_update=[])))
                    ins_.sync_info = mybir.SyncInfo(
                        on_wait=rest, on_update=list(si.on_update))
                    n_split += 1
                newlist.append(ins_)
            bb.instructions[:] = newlist
    return n_split


def _prep(edge_index):
    """Sort/shard/tile edges. Returns per-core index arrays + block tile counts."""
    src = np.concatenate([np.asarray(edge_index[0]), np.arange(N, dtype=np.int64)])
    dst = np.concatenate([np.asarray(edge_index[1]), np.arange(N, dtype=np.int64)])
    order = np.argsort(dst, kind="stable")
    src = src[order].astype(np.int64)
    dst = dst[order].astype(np.int64)

    core_of = dst // NPC
    per_core = []
    counts = np.zeros((C, NB), np.int64)
    for c in range(C):
        m = core_of == c
        s, d = src[m], dst[m]
        loc = d - c * NPC
        blk = loc // P
        np.add.at(counts[c], blk, 1)
        per_core.append((s, loc, blk))

    TB = np.maximum(1, (counts.max(axis=0) + P - 1) // P)
    TT = int(TB.sum())
    tile_starts = np.zeros(NB + 1, np.int64)
    tile_starts[1:] = np.cumsum(TB)

    srcT1 = np.zeros((C, TT, P), np.int32)
    srcT2 = np.zeros((C, TT, P), np.int32)
    dstloc = np.zeros((C, TT, P), np.int32)

    for c in range(C):
        s, loc, blk = per_core[c]
        for b in range(NB):
            m = blk == b
            sb_, lb = s[m], loc[m] - b * P
            n = len(sb_)
            cap = int(TB[b]) * P
            own = sb_ // NPC
            ln = sb_ - own * NPC
            s1 = np.full(cap, NPC, np.int64)        # core0 stripe pad (a_src=-1e9)
            s1[:n] = own * PADN + ln
            # chunked allgather layout: chunk k = ln//896 holds 8 stripes of 896
            s2 = np.full(cap, 6 * 7168 + 874, np.int64)   # core0 pad row (ln=6250)
            s2[:n] = (ln // 896) * 7168 + own * 896 + (ln % 896)
            dl = np.zeros(cap, np.int64)            # pad edges hit col 0, ex=0
            dl[:n] = lb
            t0 = tile_starts[b]
            srcT1[c, t0:t0 + TB[b]] = s1.reshape(int(TB[b]), P)
            srcT2[c, t0:t0 + TB[b]] = s2.reshape(int(TB[b]), P)
            dstloc[c, t0:t0 + TB[b]] = dl.reshape(int(TB[b]), P)

    return srcT1, srcT2, dstloc, TB, TT, tile_starts


def _build(TT, TB, tile_starts):
    """Build the SPMD Bass graph (identical for all cores)."""
    TTg = ((TT + CHT - 1) // CHT) * CHT
    NCBF = 128 + 128 + 8 + 16 + 128 + 2 + 128 + TT + 128
    NCF = 128 + 16 + ROW1 + ROW2 + 128
    tsl = [int(t) for t in tile_starts]
    blk_of_tile = np.zeros(TT, np.int64)
    for b in range(NB):
        blk_of_tile[tsl[b]:tsl[b + 1]] = b

    nc = bass.Bass()
    x_own = nc.declare_dram_parameter("x_own", [P, PADN], bf16, isOutput=False)
    idxs = nc.declare_dram_parameter("idxs", [P, 3 * TT + P + 1], i32, isOutput=False)
    dstrow = nc.declare_dram_parameter("dstrow", [1, TTg * P], bf16, isOutput=False)
    cbf = nc.declare_dram_parameter("cbf", [P, NCBF], bf16, isOutput=False)
    cf = nc.declare_dram_parameter("cf", [1, NCF], f32, isOutput=False)
    out_d = nc.declare_dram_parameter("out", [NPC, OUT_CH], f32, isOutput=True)

    ltab1 = nc.dram_tensor("ltab1", [PADN, ROW1], f32)
    tab1 = nc.dram_tensor("tab1", [C * PADN, ROW1], f32, addr_space="Shared")
    ltab2 = nc.dram_tensor("ltab2", [PADN, ROW2], f32)
    tab2 = nc.dram_tensor("tab2", [C * PADN, ROW2], f32, addr_space="Shared")

    AL = mybir.AluOpType
    AF = mybir.ActivationFunctionType

    with tile.TileContext(nc) as tc:
        from contextlib import ExitStack
        with ExitStack() as ctx:
            cpool = ctx.enter_context(tc.tile_pool(name="const", bufs=1))

            # ---- constants ----
            ci = cpool.tile([P, 3 * TT + P + 1], i32)
            nc.sync.dma_start(out=ci[:], in_=idxs[:])
            srcT1_sb = ci[:, 0:TT]
            srcT2_sb = ci[:, TT:2 * TT]
            dstlocT_sb = ci[:, 2 * TT:3 * TT]
            iota_row = ci[:, 3 * TT:3 * TT + P]
            iota_col = ci[:, 3 * TT + P:3 * TT + P + 1]

            cb = cpool.tile([P, NCBF], bf16)
            nc.sync.dma_start(out=cb[:], in_=cbf[:])
            W1_sb = cb[:, 0:128]
            W1T_sb = cb[:, 128:256]
            ablk_sb = cb[:, 256:264]
            W2_sb = cb[:, 264:280]
            W2T_sb = cb[0:16, 280:408]
            att2T_sb = cb[0:16, 408:410]
            ones_bf = cb[0:1, 410:538]  # ones row [1, 128]
            dstloc_bf = cb[:, 538:538 + TT]
            iota_row_bf = cb[:, 538 + TT:538 + TT + P]

            cfs = cpool.tile([1, NCF], f32)
            nc.sync.dma_start(out=cfs[:], in_=cf[:])
            b1_row = cfs[:, 0:128]
            b2_row = cfs[:, 128:144]
            pad1_row = cfs[:, 144:144 + ROW1]
            pad2_row = cfs[:, 144 + ROW1:144 + ROW1 + ROW2]
            ones_f = cfs[:, 144 + ROW1 + ROW2:NCF]  # ones row [1, 128]

            xo = cpool.tile([P, PADN], bf16)
            nc.sync.dma_start(out=xo[:], in_=x_own[:])

            # prime DVE on const DMAs (keeps later waits <= 1 per instruction)
            pr = cpool.tile([P, 1], f32)
            nc.vector.tensor_tensor(out=pr[:], in0=ci[:, 0:1], in1=cb[:, 0:1],
                                    op=AL.add)
            nc.vector.tensor_tensor(out=pr[0:1, :], in0=cfs[0:1, 0:1],
                                    in1=xo[0:1, 0:1], op=AL.add)

            adst1 = cpool.tile([P, NB * HEADS], bf16)
            adst2 = cpool.tile([P, NB], bf16)

            from concourse.masks import make_identity
            ident = cpool.tile([P, P], f32)
            make_identity(nc, ident[:])

            with tc.tile_pool(name="ps0", bufs=2, space="PSUM") as ps0:
                # W1cat = [W1 | W1@ablk_src | W1@ablk_dst] bf16
                w1cat = cpool.tile([P, 136], bf16)
                nc.vector.tensor_copy(out=w1cat[:, 0:128], in_=W1_sb)
                ps_w = ps0.tile([P, 8], f32, tag="pw")
                nc.tensor.matmul(out=ps_w[:], lhsT=W1T_sb, rhs=ablk_sb,
                                 start=True, stop=True)
                nc.vector.tensor_copy(out=w1cat[:, 128:136], in_=ps_w[:])

                # vsd = W2 @ [att_src2.T | att_dst2.T] -> [128, 2] bf16
                vsd = cpool.tile([P, 2], bf16)
                ps_v = ps0.tile([P, 2], f32, tag="pw")
                nc.tensor.matmul(out=ps_v[:], lhsT=W2T_sb, rhs=att2T_sb,
                                 start=True, stop=True)
                nc.vector.tensor_copy(out=vsd[:], in_=ps_v[:])

                # bias broadcast rows -> [128, *] f32
                b1bc = cpool.tile([P, 128], f32)
                ps_b = ps0.tile([P, 128], f32, tag="pw")
                nc.tensor.matmul(out=ps_b[:], lhsT=ones_f, rhs=b1_row,
                                 start=True, stop=True)
                nc.vector.tensor_copy(out=b1bc[:], in_=ps_b[:])
                b2bc = cpool.tile([P, 16], f32)
                ps_b2 = ps0.tile([P, 16], f32, tag="pw")
                nc.tensor.matmul(out=ps_b2[:], lhsT=ones_f, rhs=b2_row,
                                 start=True, stop=True)
                nc.vector.tensor_copy(out=b2bc[:], in_=ps_b2[:])

                # pad rows
                nc.sync.dma_start(out=ltab1[NPC:NPC + 1, :], in_=pad1_row)
                # only row NPC is ever gathered (the pad target); rows
                # NPC+1..PADN are never referenced
                nc.sync.dma_start(out=ltab2[NPC:NPC + 1, :], in_=pad2_row)

                # ---- phase 1: own 49 blocks of h1/a_src1 from x_own, then
                # AllGather the stripe table; a_dst1 for own nodes ----
                with nc.named_scope("phase1"), \
                     tc.tile_pool(name="p1", bufs=4) as p1:
                    for b in range(NB):
                        ph = ps0.tile([P, 136], f32, tag="ph")
                        nc.tensor.matmul(out=ph[:],
                                         lhsT=xo[:, b * P:(b + 1) * P],
                                         rhs=w1cat[:], start=True, stop=True)
                        pk = p1.tile([P, ROW1], f32, tag="pk")
                        nc.vector.tensor_copy(out=pk[:, 0:64].bitcast(bf16),
                                              in_=ph[:, 0:128])
                        nc.vector.tensor_copy(out=pk[:, 64:68],
                                              in_=ph[:, 128:132])
                        rows = min(P, NPC - b * P)
                        nc.sync.dma_start(out=ltab1[b * P:b * P + rows, :],
                                          in_=pk[0:rows, :])
                    nc.gpsimd.collective_compute(
                        "AllGather", mybir.AluOpType.bypass,
                        replica_groups=[list(range(C))],
                        ins=[ltab1[:]], outs=[tab1[:]])
                    for b in range(NB):
                        pa = ps0.tile([P, 4], f32, tag="pa")
                        nc.tensor.matmul(out=pa[:],
                                         lhsT=xo[:, b * P:(b + 1) * P],
                                         rhs=w1cat[:, 132:136],
                                         start=True, stop=True)
                        nc.vector.tensor_copy(out=adst1[:, b * 4:(b + 1) * 4],
                                              in_=pa[:])

            # ---- edge phases ----
            sbp = ctx.enter_context(tc.tile_pool(name="sbp", bufs=4))
            gpo = ctx.enter_context(tc.tile_pool(name="gpo", bufs=8))
            drp = ctx.enter_context(tc.tile_pool(name="drp", bufs=2))
            psE = ctx.enter_context(tc.tile_pool(name="psE", bufs=2, space="PSUM"))
            psF = ctx.enter_context(tc.tile_pool(name="psF", bufs=2, space="PSUM"))

            def edge_phase(layer):
                tabsrc = tab1 if layer == 1 else tab2
                srcsb = srcT1_sb if layer == 1 else srcT2_sb
                rowlen = ROW1 if layer == 1 else ROW2
                nh = HEADS if layer == 1 else 1
                mcols = 132 if layer == 1 else 129
                adst = adst1 if layer == 1 else adst2
                drow_t = None
                pb = None
                for g0 in range(0, TT, GRP):
                    ng = min(GRP, TT - g0)
                    if g0 % CHT == 0:
                        drow_t = drp.tile([1, CHT * P], bf16, tag="drow")
                        nc.sync.dma_start(out=drow_t[:],
                                          in_=dstrow[:, g0 * P:(g0 + CHT) * P])
                    off = (g0 % CHT) * P
                    pbc = psE.tile([P, GRP * P], f32, tag="pbc")
                    nc.tensor.matmul(out=pbc[:, 0:ng * P], lhsT=ones_bf,
                                     rhs=drow_t[:, off:off + ng * P],
                                     start=True, stop=True)
                    oht4 = sbp.tile([P, GRP * P], bf16, tag="oht4")
                    nc.vector.tensor_tensor(
                        out=oht4[:, 0:ng * P],
                        in0=iota_col.to_broadcast([P, ng * P]),
                        in1=pbc[:, 0:ng * P], op=AL.is_equal)
                    oh4 = sbp.tile([P, GRP * P], bf16, tag="oh4")
                    nc.vector.tensor_tensor(
                        out=oh4[:, 0:ng * P].rearrange("p (g j) -> p g j", g=ng),
                        in0=dstloc_bf[:, g0:g0 + ng]
                            .rearrange("p (g o) -> p g o", o=1)
                            .to_broadcast([P, ng, P]),
                        in1=iota_row_bf.rearrange("p (o j) -> p o j", o=1)
                            .to_broadcast([P, ng, P]),
                        op=AL.is_equal)
                    # gathers for the whole group into one buffer
                    g4 = gpo.tile([P, GRP * rowlen], f32, tag="g4")
                    for j in range(ng):
                        nc.gpsimd.indirect_dma_start(
                            out=g4[:, j * rowlen:(j + 1) * rowlen],
                            out_offset=None, in_=tabsrc[:],
                            in_offset=bass.IndirectOffsetOnAxis(
                                ap=srcsb[:, g0 + j:g0 + j + 1], axis=0))
                    # a_dst per edge via one-hot matmuls (bf16)
                    pD16 = psE.tile([P, GRP * 4], f32, tag="pD")
                    for j in range(ng):
                        bj = int(blk_of_tile[g0 + j])
                        nc.tensor.matmul(out=pD16[:, j * 4:j * 4 + nh],
                                         lhsT=oht4[:, j * P:(j + 1) * P],
                                         rhs=adst[:, bj * nh:(bj + 1) * nh],
                                         start=True, stop=True)
                    # e = a_src + a_dst, leaky, exp -- batched over the group
                    e16 = gpo.tile([P, GRP * 4], f32, tag="e16")
                    nc.vector.tensor_tensor(
                        out=e16[:, 0:ng * 4].rearrange("p (g h) -> p g h", g=ng)[:, :, 0:nh],
                        in0=g4[:, 0:ng * rowlen]
                            .rearrange("p (g r) -> p g r", g=ng)[:, :, 64:64 + nh],
                        in1=pD16[:, 0:ng * 4]
                            .rearrange("p (g h) -> p g h", g=ng)[:, :, 0:nh],
                        op=AL.add)
                    t16 = gpo.tile([P, GRP * 4], f32, tag="t16")
                    nc.vector.tensor_scalar_mul(out=t16[:, 0:ng * 4],
                                                in0=e16[:, 0:ng * 4], scalar1=NEG)
                    l16 = gpo.tile([P, GRP * 4], f32, tag="l16")
                    nc.vector.tensor_tensor(out=l16[:, 0:ng * 4],
                                            in0=e16[:, 0:ng * 4],
                                            in1=t16[:, 0:ng * 4], op=AL.max)
                    m4 = gpo.tile([P, GRP * 132], bf16, tag="m4")
                    nc.scalar.activation(
                        out=m4[:, 0:ng * 132]
                            .rearrange("p (g r) -> p g r", g=ng)[:, :, 128:132],
                        in_=l16[:, 0:ng * 4].rearrange("p (g h) -> p g h", g=ng),
                        func=AF.Exp)
                    g4b = g4[:, 0:ng * rowlen].bitcast(bf16)   # [P, ng*rowlen*2]
                    for j in range(ng):
                        nc.vector.tensor_tensor(
                            out=m4[:, j * 132:j * 132 + 128]
                                .rearrange("p (h c) -> p h c", h=nh),
                            in0=g4b[:, j * rowlen * 2:j * rowlen * 2 + 128]
                                .rearrange("p (h c) -> p h c", h=nh),
                            in1=m4[:, j * 132 + 128:j * 132 + 128 + nh]
                                .rearrange("p (h o) -> p h o", o=1)
                                .to_broadcast([P, nh, 128 // nh]),
                            op=AL.mult)
                    for j in range(ng):
                        gt = g0 + j
                        b = int(blk_of_tile[gt])
                        first = gt == tsl[b]
                        last = gt == tsl[b + 1] - 1
                        if first:
                            pb = psF.tile([P, mcols], f32, tag="pb")
                        nc.tensor.matmul(out=pb[:],
                                         lhsT=oh4[:, j * P:(j + 1) * P],
                                         rhs=m4[:, j * 132:j * 132 + mcols],
                                         start=first, stop=last)
                        if last:
                            finalize(layer, b, pb)

            def finalize(layer, b, pb):
                rows = min(P, NPC - b * P)
                nh = HEADS if layer == 1 else 1
                den = gpo.tile([P, 4], f32, tag="den")
                nc.vector.tensor_scalar_add(out=den[:, 0:nh],
                                            in0=pb[:, 128:128 + nh],
                                            scalar1=1e-16)
                rec = gpo.tile([P, 4], f32, tag="rec")
                nc.vector.reciprocal(out=rec[:, 0:nh], in_=den[:, 0:nh])
                if layer == 1:
                    tmp = sbp.tile([P, 128], f32, tag="tmp")
                    nc.vector.tensor_tensor(
                        out=tmp[:].rearrange("p (h c) -> p h c", h=nh),
                        in0=pb[:, 0:128].rearrange("p (h c) -> p h c", h=nh),
                        in1=rec[:, 0:nh].rearrange("p (h o) -> p h o", o=1)
                            .to_broadcast([P, nh, 128 // nh]),
                        op=AL.mult)
                    nc.vector.tensor_tensor(out=tmp[:], in0=tmp[:], in1=b1bc[:],
                                            op=AL.add)
                    nc.vector.tensor_scalar_max(out=tmp[:], in0=tmp[:], scalar1=0.0)
                    upk = sbp.tile([P, ROW1], f32, tag="upk")  # ROW1>=ROW2
                    nc.vector.tensor_copy(out=upk[:, 0:64].bitcast(bf16), in_=tmp[:])
                    pt = psF.tile([P, P], f32, tag="pt")
                    nc.tensor.transpose(out=pt[:], in_=tmp[:], identity=ident[:])
                    uT = sbp.tile([P, P], bf16, tag="uT")
                    nc.vector.tensor_copy(out=uT[:], in_=pt[:])
                    pa2 = psE.tile([P, 2], f32, tag="pD")
                    nc.tensor.matmul(out=pa2[:], lhsT=uT[:], rhs=vsd[:],
                                     start=True, stop=True)
                    nc.vector.tensor_copy(out=upk[:, 64:65], in_=pa2[:, 0:1])
                    nc.vector.tensor_copy(out=adst2[:, b:b + 1], in_=pa2[:, 1:2])
                    nc.sync.dma_start(out=ltab2[b * P:b * P + rows, :],
                                      in_=upk[0:rows, 0:ROW2])
                    if b % 7 == 6:
                        k = b // 7
                        nc.gpsimd.collective_compute(
                            "AllGather", mybir.AluOpType.bypass,
                            replica_groups=[list(range(C))],
                            ins=[ltab2[k * 896:(k + 1) * 896, :]],
                            outs=[tab2[k * 7168:(k + 1) * 7168, :]])
                else:
                    agg = sbp.tile([P, P], f32, tag="tmp")
                    nc.vector.tensor_scalar_mul(out=agg[:], in0=pb[:, 0:128],
                                                scalar1=rec[:, 0:1])
                    pt = psF.tile([P, P], f32, tag="pt")
                    nc.tensor.transpose(out=pt[:], in_=agg[:], identity=ident[:])
                    aT = sbp.tile([P, P], bf16, tag="uT")
                    nc.vector.tensor_copy(out=aT[:], in_=pt[:])
                    pz = psE.tile([P, 16], f32, tag="pD")
                    nc.tensor.matmul(out=pz[:], lhsT=aT[:], rhs=W2_sb,
                                     start=True, stop=True)
                    z = gpo.tile([P, 16], f32, tag="z")
                    nc.vector.tensor_tensor(out=z[:], in0=pz[:], in1=b2bc[:],
                                            op=AL.add)
                    mx = gpo.tile([P, 1], f32, tag="mx")
                    nc.vector.tensor_reduce(out=mx[:], in_=z[:],
                                            axis=mybir.AxisListType.X,
                                            op=AL.max, negate=True)  # -max
                    es = gpo.tile([P, 16], f32, tag="es")
                    ssum = gpo.tile([P, 1], f32, tag="ssum")
                    nc.scalar.activation(out=es[:], in_=z[:], func=AF.Exp,
                                         bias=mx[:], accum_out=ssum[:])
                    ls = gpo.tile([P, 1], f32, tag="ls")
                    nc.scalar.activation(out=ls[:], in_=ssum[:], func=AF.Ln)
                    sh = gpo.tile([P, 1], f32, tag="sh")
                    nc.vector.tensor_tensor(out=sh[:], in0=ls[:], in1=mx[:],
                                            op=AL.subtract)  # ln(s) - (-max)... see note
                    res = gpo.tile([P, 16], f32, tag="res")
                    nc.vector.tensor_scalar_sub(out=res[:], in0=z[:],
                                                scalar1=sh[:, 0:1])
                    nc.sync.dma_start(out=out_d[b * P:b * P + rows, :],
                                      in_=res[0:rows, :])

            with nc.named_scope("edge1"):
                edge_phase(1)

            with nc.named_scope("edge2"):
                edge_phase(2)

    _split_excess_waits(nc)
    return nc


# log_softmax shift note: out = z - max - ln(sum(exp(z - max))).
# mx holds -max (negate=True). es = exp(z + mx), ssum = sum(es), ls = ln(ssum).
# shift = max + ls = ls - mx. res = z - shift.


def _host_arrays(x, W1, att_src1, att_dst1, b1, W2, att_src2, att_dst2, b2,
                 srcT1, srcT2, dstloc, TT):
    TTg = ((TT + CHT - 1) // CHT) * CHT
    xT = np.zeros((P, XPAD), bfnp)
    xT[:, 0:N] = np.asarray(x, np.float32).T.astype(bfnp)

    x_own = np.zeros((C, P, PADN), bfnp)
    for c in range(C):
        end = min(c * NPC + PADN, XPAD)
        x_own[c, :, 0:end - c * NPC] = xT[:, c * NPC:end]

    iota_row = np.broadcast_to(np.arange(P, dtype=np.int32)[None, :], (P, P))
    iota_col = np.arange(P, dtype=np.int32).reshape(P, 1)
    idxs = np.zeros((C, P, 3 * TT + P + 1), np.int32)
    for c in range(C):
        idxs[c, :, 0:TT] = srcT1[c].T
        idxs[c, :, TT:2 * TT] = srcT2[c].T
        idxs[c, :, 2 * TT:3 * TT] = dstloc[c].T
        idxs[c, :, 3 * TT:3 * TT + P] = iota_row
        idxs[c, :, 3 * TT + P:] = iota_col

    dstrow = np.zeros((C, 1, TTg * P), bfnp)
    for c in range(C):
        dstrow[c, 0, 0:TT * P] = dstloc[c].reshape(-1).astype(bfnp)

    NCBF = 128 + 128 + 8 + 16 + 128 + 2 + 128 + TT + 128
    cbf = np.zeros((C, P, NCBF), bfnp)
    W1f = np.asarray(W1, np.float32)
    cbf[:, :, 0:128] = W1f.astype(bfnp)
    cbf[:, :, 128:256] = W1f.T.astype(bfnp)
    ablk = np.zeros((128, 8), np.float32)
    for h in range(HEADS):
        ablk[h * HID:(h + 1) * HID, h] = np.asarray(att_src1, np.float32)[h]
        ablk[h * HID:(h + 1) * HID, 4 + h] = np.asarray(att_dst1, np.float32)[h]
    cbf[:, :, 256:264] = ablk.astype(bfnp)
    W2f = np.asarray(W2, np.float32)
    cbf[:, :, 264:280] = W2f.astype(bfnp)
    cbf[:, 0:16, 280:408] = W2f.T.astype(bfnp)
    cbf[:, 0:16, 408:409] = np.asarray(att_src2, np.float32).T.astype(bfnp)
    cbf[:, 0:16, 409:410] = np.asarray(att_dst2, np.float32).T.astype(bfnp)
    cbf[:, 0:1, 410:538] = np.ones((1, 128), bfnp)
    for c in range(C):
        cbf[c, :, 538:538 + TT] = np.ascontiguousarray(dstloc[c].T).astype(bfnp)
    cbf[:, :, 538 + TT:538 + TT + 128] = np.broadcast_to(
        np.arange(128, dtype=np.float32)[None, None, :], (C, P, 128)).astype(bfnp)

    NCF = 128 + 16 + ROW1 + ROW2 + 128
    cf = np.zeros((1, NCF), np.float32)
    cf[0, 0:128] = np.asarray(b1, np.float32)
    cf[0, 128:144] = np.asarray(b2, np.float32)
    pad1 = np.zeros(ROW1, np.float32)
    pad1[64:68] = -1e9
    cf[0, 144:144 + ROW1] = pad1
    pad2 = np.zeros(ROW2, np.float32)
    pad2[64] = -1e9
    cf[0, 144 + ROW1:144 + ROW1 + ROW2] = pad2
    cf[0, 144 + ROW1 + ROW2:NCF] = 1.0

    return xT, x_own, idxs, dstrow, cbf, cf


_CACHE = {}


def kernel(x, edge_index, W1, att_src1, att_dst1, b1, W2, att_src2, att_dst2, b2,
           _trace=False, _tmpdir=None):
    srcT1, srcT2, dstloc, TB, TT, tile_starts = _prep(np.asarray(edge_index))
    xT, x_own, idxs, dstrow, cbf, cf = _host_arrays(
        x, W1, att_src1, att_dst1, b1, W2, att_src2, att_dst2, b2,
        srcT1, srcT2, dstloc, TT)

    key = (TT, tuple(int(t) for t in TB))
    if key not in _CACHE:
        _CACHE[key] = _build(TT, TB, tile_starts)
    nc = _CACHE[key]

    in_maps = []
    for c in range(C):
        in_maps.append({
            "x_own": np.ascontiguousarray(x_own[c]),
            "idxs": np.ascontiguousarray(idxs[c]),
            "dstrow": np.ascontiguousarray(dstrow[c]),
            "cbf": np.ascontiguousarray(cbf[c]), "cf": cf,
        })

    res = run_bass_kernel_spmd(nc, in_maps, list(range(C)), trace=_trace,
                               tmpdir=_tmpdir)
    out = np.concatenate([res.results[c]["out"] for c in range(C)], axis=0)
    kernel.last_results = res
    return out.astype(np.float32)


# revision 16
# speedup vs baseline: 1.6279x; 1.0066x over previous
"""GAT (2-layer, PyG-style) Trainium2 kernel, edge-parallel across 8 NeuronCores.

Self-contained: host-side numpy preprocessing (sharding / edge sorting / index
tables), Bass/Tile kernel build, SPMD execution on cores 0-7, gather of the
full [50000, 16] log-softmax output.

Strategy:
  - edges (plus self loops) sorted by dst; dst-range sharded: core d owns dst in
    [d*6250, (d+1)*6250) so all segment reductions are core-local.
  - per 128-node block, edges are tiled into 128-edge tiles; segment softmax
    sums are one-hot matmuls accumulated in PSUM (no scatter).
  - softmax without max subtraction (logits are O(1) for this model; verified
    offline: max |e| < 4, so exp never overflows; self loops keep denom > 0).
  - layer-2 messages: W2 is linear and applied after aggregation (heads=1), so
    aggregate relu(h1) with layer-2 attention weights first, then matmul W2.
  - node tables replicated: each core computes the full [N] layer-1 table; the
    layer-2 table is AllGathered (each core produces its own 6250 rows).
"""
import numpy as np
import ml_dtypes

import concourse.bass as bass
import concourse.mybir as mybir
import concourse.tile as tile
from concourse.bass_utils import run_bass_kernel_spmd

bfnp = ml_dtypes.bfloat16

N = 50000
E = 800000
IN_CH = 128
HID = 32
HEADS = 4
OUT_CH = 16
NEG = 0.2
C = 8
NPC = N // C               # 6250 nodes per core
P = 128
NB = (NPC + P - 1) // P    # 49 blocks per core
PADN = NB * P              # 6272 rows per core stripe in allgathered table
ROW1 = 68                  # tab1 row: 64 f32 words (128 bf16 h) + 4 f32 a_src
ROW2 = 65                  # tab2 row: 64 f32 words (128 bf16 u) + 1 f32 a_src2
GRP = 4                    # one-hot generation group (one 512-wide psum bank)
CHT = 64                   # dstrow chunk length in tiles
NT1 = (N + P - 1) // P     # 391 node tiles (50048 padded)
XPAD = NT1 * P

f32 = mybir.dt.float32
bf16 = mybir.dt.bfloat16
i32 = mybir.dt.int32


def _split_excess_waits(nc, keep=1):
    """TRN2 walrus codegen rejects instructions carrying more than one
    sync-wait command; hoist extras onto same-engine NoOp carriers."""
    _skip = (mybir.InstEventSemaphore,)
    n_split = 0
    for fn in nc.m.functions:
        for bb in fn.blocks:
            newlist = []
            for ins_ in bb.instructions:
                si = ins_.sync_info
                if (si is not None and si.on_wait and len(si.on_wait) > keep
                        and not isinstance(ins_, _skip)):
                    waits = list(si.on_wait)
                    extra, rest = waits[:-keep], waits[-keep:]
                    for j, w in enumerate(extra):
                        newlist.append(mybir.InstNoOp(
                            name=f"{ins_.name}-wn{j}", engine=ins_.engine,
                            ins=[], outs=[],
                            sync_info=mybir.SyncInfo(on_wait=[w], on_update=[])))
                    ins_.sync_info = mybir.SyncInfo(
                        on_wait=rest, on_update=list(si.on_update))
                    n_split += 1
                newlist.append(ins_)
            bb.instructions[:] = newlist
    return n_split


def _prep(edge_index):
    """Sort/shard/tile edges. Returns per-core index arrays + block tile counts."""
    src = np.concatenate([np.asarray(edge_index[0]), np.arange(N, dtype=np.int64)])
    dst = np.concatenate([np.asarray(edge_index[1]), np.arange(N, dtype=np.int64)])
    order = np.argsort(dst, kind="stable")
    src = src[order].astype(np.int64)
    dst = dst[order].astype(np.int64)

    core_of = dst // NPC
    per_core = []
    counts = np.zeros((C, NB), np.int64)
    for c in range(C):
        m = core_of == c
        s, d = src[m], dst[m]
        loc = d - c * NPC
        blk = loc // P
        np.add.at(counts[c], blk, 1)
        per_core.append((s, loc, blk))

    TB = np.maximum(1, (counts.max(axis=0) + P - 1) // P)
    TT = int(TB.sum())
    tile_starts = np.zeros(NB + 1, np.int64)
    tile_starts[1:] = np.cumsum(TB)

    srcT1 = np.zeros((C, TT, P), np.int32)
    srcT2 = np.zeros((C, TT, P), np.int32)
    dstloc = np.zeros((C, TT, P), np.int32)

    for c in range(C):
        s, loc, blk = per_core[c]
        for b in range(NB):
            m = blk == b
            sb_, lb = s[m], loc[m] - b * P
            n = len(sb_)
            cap = int(TB[b]) * P
            own = sb_ // NPC
            ln = sb_ - own * NPC
            s1 = np.full(cap, NPC, np.int64)        # core0 stripe pad (a_src=-1e9)
            s1[:n] = own * PADN + ln
            # chunked allgather layout: chunk k = ln//896 holds 8 stripes of 896
            s2 = np.full(cap, 6 * 7168 + 874, np.int64)   # core0 pad row (ln=6250)
            s2[:n] = (ln // 896) * 7168 + own * 896 + (ln % 896)
            dl = np.zeros(cap, np.int64)            # pad edges hit col 0, ex=0
            dl[:n] = lb
            t0 = tile_starts[b]
            srcT1[c, t0:t0 + TB[b]] = s1.reshape(int(TB[b]), P)
            srcT2[c, t0:t0 + TB[b]] = s2.reshape(int(TB[b]), P)
            dstloc[c, t0:t0 + TB[b]] = dl.reshape(int(TB[b]), P)

    return srcT1, srcT2, dstloc, TB, TT, tile_starts


def _build(TT, TB, tile_starts):
    """Build the SPMD Bass graph (identical for all cores)."""
    TTg = ((TT + CHT - 1) // CHT) * CHT
    NCBF = 128 + 128 + 8 + 16 + 128 + 2 + 128 + TT + 128
    NCF = 128 + 16 + ROW1 + ROW2 + 128
    tsl = [int(t) for t in tile_starts]
    blk_of_tile = np.zeros(TT, np.int64)
    for b in range(NB):
        blk_of_tile[tsl[b]:tsl[b + 1]] = b

    nc = bass.Bass()
    x_own = nc.declare_dram_parameter("x_own", [P, PADN], bf16, isOutput=False)
    idxs = nc.declare_dram_parameter("idxs", [P, 3 * TT + P + 1], i32, isOutput=False)
    dstrow = nc.declare_dram_parameter("dstrow", [1, TTg * P], bf16, isOutput=False)
    cbf = nc.declare_dram_parameter("cbf", [P, NCBF], bf16, isOutput=False)
    cf = nc.declare_dram_parameter("cf", [1, NCF], f32, isOutput=False)
    out_d = nc.declare_dram_parameter("out", [NPC, OUT_CH], f32, isOutput=True)

    ltab1 = nc.dram_tensor("ltab1", [PADN, ROW1], f32)
    tab1 = nc.dram_tensor("tab1", [C * PADN, ROW1], f32, addr_space="Shared")
    ltab2 = nc.dram_tensor("ltab2", [PADN, ROW2], f32)
    tab2 = nc.dram_tensor("tab2", [C * PADN, ROW2], f32, addr_space="Shared")

    AL = mybir.AluOpType
    AF = mybir.ActivationFunctionType

    with tile.TileContext(nc) as tc:
        from contextlib import ExitStack
        with ExitStack() as ctx:
            cpool = ctx.enter_context(tc.tile_pool(name="const", bufs=1))

            # ---- constants ----
            ci = cpool.tile([P, 3 * TT + P + 1], i32)
            nc.sync.dma_start(out=ci[:], in_=idxs[:])
            srcT1_sb = ci[:, 0:TT]
            srcT2_sb = ci[:, TT:2 * TT]
            dstlocT_sb = ci[:, 2 * TT:3 * TT]
            iota_row = ci[:, 3 * TT:3 * TT + P]
            iota_col = ci[:, 3 * TT + P:3 * TT + P + 1]

            cb = cpool.tile([P, NCBF], bf16)
            nc.sync.dma_start(out=cb[:], in_=cbf[:])
            W1_sb = cb[:, 0:128]
            W1T_sb = cb[:, 128:256]
            ablk_sb = cb[:, 256:264]
            W2_sb = cb[:, 264:280]
            W2T_sb = cb[0:16, 280:408]
            att2T_sb = cb[0:16, 408:410]
            ones_bf = cb[0:1, 410:538]  # ones row [1, 128]
            dstloc_bf = cb[:, 538:538 + TT]
            iota_row_bf = cb[:, 538 + TT:538 + TT + P]

            cfs = cpool.tile([1, NCF], f32)
            nc.sync.dma_start(out=cfs[:], in_=cf[:])
            b1_row = cfs[:, 0:128]
            b2_row = cfs[:, 128:144]
            pad1_row = cfs[:, 144:144 + ROW1]
            pad2_row = cfs[:, 144 + ROW1:144 + ROW1 + ROW2]
            ones_f = cfs[:, 144 + ROW1 + ROW2:NCF]  # ones row [1, 128]

            xo = cpool.tile([P, PADN], bf16)
            nc.sync.dma_start(out=xo[:], in_=x_own[:])

            # prime DVE on const DMAs (keeps later waits <= 1 per instruction)
            pr = cpool.tile([P, 1], f32)
            nc.vector.tensor_tensor(out=pr[:], in0=ci[:, 0:1], in1=cb[:, 0:1],
                                    op=AL.add)
            nc.vector.tensor_tensor(out=pr[0:1, :], in0=cfs[0:1, 0:1],
                                    in1=xo[0:1, 0:1], op=AL.add)

            adst1 = cpool.tile([P, NB * HEADS], bf16)
            adst2 = cpool.tile([P, NB], bf16)

            from concourse.masks import make_identity
            ident = cpool.tile([P, P], f32)
            make_identity(nc, ident[:])

            with tc.tile_pool(name="ps0", bufs=2, space="PSUM") as ps0:
                # W1cat = [W1 | W1@ablk_src | W1@ablk_dst] bf16
                w1cat = cpool.tile([P, 136], bf16)
                nc.vector.tensor_copy(out=w1cat[:, 0:128], in_=W1_sb)
                ps_w = ps0.tile([P, 8], f32, tag="pw")
                nc.tensor.matmul(out=ps_w[:], lhsT=W1T_sb, rhs=ablk_sb,
                                 start=True, stop=True)
                nc.vector.tensor_copy(out=w1cat[:, 128:136], in_=ps_w[:])

                # vsd = W2 @ [att_src2.T | att_dst2.T] -> [128, 2] bf16
                vsd = cpool.tile([P, 2], bf16)
                ps_v = ps0.tile([P, 2], f32, tag="pw")
                nc.tensor.matmul(out=ps_v[:], lhsT=W2T_sb, rhs=att2T_sb,
                                 start=True, stop=True)
                nc.vector.tensor_copy(out=vsd[:], in_=ps_v[:])

                # bias broadcast rows -> [128, *] f32
                b1bc = cpool.tile([P, 128], f32)
                ps_b = ps0.tile([P, 128], f32, tag="pw")
                nc.tensor.matmul(out=ps_b[:], lhsT=ones_f, rhs=b1_row,
                                 start=True, stop=True)
                nc.vector.tensor_copy(out=b1bc[:], in_=ps_b[:])
                b2bc = cpool.tile([P, 16], f32)
                ps_b2 = ps0.tile([P, 16], f32, tag="pw")
                nc.tensor.matmul(out=ps_b2[:], lhsT=ones_f, rhs=b2_row,
                                 start=True, stop=True)
                nc.vector.tensor_copy(out=b2bc[:], in_=ps_b2[:])

                # pad rows
                nc.sync.dma_start(out=ltab1[NPC:NPC + 1, :], in_=pad1_row)
                # only row NPC is ever gathered (the pad target); rows
                # NPC+1..PADN are never referenced
                nc.sync.dma_start(out=ltab2[NPC:NPC + 1, :], in_=pad2_row)

                # ---- phase 1: own 49 blocks of h1/a_src1 from x_own, then
                # AllGather the stripe table; a_dst1 for own nodes ----
                with nc.named_scope("phase1"), \
                     tc.tile_pool(name="p1", bufs=4) as p1:
                    for b in range(NB):
                        ph = ps0.tile([P, 136], f32, tag="ph")
                        nc.tensor.matmul(out=ph[:],
                                         lhsT=xo[:, b * P:(b + 1) * P],
                                         rhs=w1cat[:], start=True, stop=True)
                        pk = p1.tile([P, ROW1], f32, tag="pk")
                        nc.vector.tensor_copy(out=pk[:, 0:64].bitcast(bf16),
                                              in_=ph[:, 0:128])
                        nc.vector.tensor_copy(out=pk[:, 64:68],
                                              in_=ph[:, 128:132])
                        rows = min(P, NPC - b * P)
                        nc.sync.dma_start(out=ltab1[b * P:b * P + rows, :],
                                          in_=pk[0:rows, :])
                    nc.gpsimd.collective_compute(
                        "AllGather", mybir.AluOpType.bypass,
                        replica_groups=[list(range(C))],
                        ins=[ltab1[:]], outs=[tab1[:]])
                    for b in range(NB):
                        pa = ps0.tile([P, 4], f32, tag="pa")
                        nc.tensor.matmul(out=pa[:],
                                         lhsT=xo[:, b * P:(b + 1) * P],
                                         rhs=w1cat[:, 132:136],
                                         start=True, stop=True)
                        nc.vector.tensor_copy(out=adst1[:, b * 4:(b + 1) * 4],
                                              in_=pa[:])

            # ---- edge phases ----
            sbp = ctx.enter_context(tc.tile_pool(name="sbp", bufs=4))
            gpo = ctx.enter_context(tc.tile_pool(name="gpo", bufs=8))
            drp = ctx.enter_context(tc.tile_pool(name="drp", bufs=2))
            g4p = ctx.enter_context(tc.tile_pool(name="g4p", bufs=16))
            psE = ctx.enter_context(tc.tile_pool(name="psE", bufs=2, space="PSUM"))
            psF = ctx.enter_context(tc.tile_pool(name="psF", bufs=2, space="PSUM"))

            def edge_phase(layer):
                tabsrc = tab1 if layer == 1 else tab2
                srcsb = srcT1_sb if layer == 1 else srcT2_sb
                rowlen = ROW1 if layer == 1 else ROW2
                nh = HEADS if layer == 1 else 1
                mcols = 132 if layer == 1 else 129
                adst = adst1 if layer == 1 else adst2
                drow_t = None
                pb = None
                for g0 in range(0, TT, GRP):
                    ng = min(GRP, TT - g0)
                    if g0 % CHT == 0:
                        drow_t = drp.tile([1, CHT * P], bf16, tag="drow")
                        nc.sync.dma_start(out=drow_t[:],
                                          in_=dstrow[:, g0 * P:(g0 + CHT) * P])
                    off = (g0 % CHT) * P
                    pbc = psE.tile([P, GRP * P], f32, tag="pbc")
                    nc.tensor.matmul(out=pbc[:, 0:ng * P], lhsT=ones_bf,
                                     rhs=drow_t[:, off:off + ng * P],
                                     start=True, stop=True)
                    oht4 = sbp.tile([P, GRP * P], bf16, tag="oht4")
                    nc.vector.tensor_tensor(
                        out=oht4[:, 0:ng * P],
                        in0=iota_col.to_broadcast([P, ng * P]),
                        in1=pbc[:, 0:ng * P], op=AL.is_equal)
                    oh4 = sbp.tile([P, GRP * P], bf16, tag="oh4")
                    nc.vector.tensor_tensor(
                        out=oh4[:, 0:ng * P].rearrange("p (g j) -> p g j", g=ng),
                        in0=dstloc_bf[:, g0:g0 + ng]
                            .rearrange("p (g o) -> p g o", o=1)
                            .to_broadcast([P, ng, P]),
                        in1=iota_row_bf.rearrange("p (o j) -> p o j", o=1)
                            .to_broadcast([P, ng, P]),
                        op=AL.is_equal)
                    # gathers for the whole group into one buffer
                    g4 = g4p.tile([P, GRP * rowlen], f32, tag="g4")
                    for j in range(ng):
                        nc.gpsimd.indirect_dma_start(
                            out=g4[:, j * rowlen:(j + 1) * rowlen],
                            out_offset=None, in_=tabsrc[:],
                            in_offset=bass.IndirectOffsetOnAxis(
                                ap=srcsb[:, g0 + j:g0 + j + 1], axis=0))
                    # a_dst per edge via one-hot matmuls (bf16)
                    pD16 = psE.tile([P, GRP * 4], f32, tag="pD")
                    for j in range(ng):
                        bj = int(blk_of_tile[g0 + j])
                        nc.tensor.matmul(out=pD16[:, j * 4:j * 4 + nh],
                                         lhsT=oht4[:, j * P:(j + 1) * P],
                                         rhs=adst[:, bj * nh:(bj + 1) * nh],
                                         start=True, stop=True)
                    # e = a_src + a_dst, leaky, exp -- batched over the group
                    e16 = gpo.tile([P, GRP * 4], f32, tag="e16")
                    nc.vector.tensor_tensor(
                        out=e16[:, 0:ng * 4].rearrange("p (g h) -> p g h", g=ng)[:, :, 0:nh],
                        in0=g4[:, 0:ng * rowlen]
                            .rearrange("p (g r) -> p g r", g=ng)[:, :, 64:64 + nh],
                        in1=pD16[:, 0:ng * 4]
                            .rearrange("p (g h) -> p g h", g=ng)[:, :, 0:nh],
                        op=AL.add)
                    t16 = gpo.tile([P, GRP * 4], f32, tag="t16")
                    nc.vector.tensor_scalar_mul(out=t16[:, 0:ng * 4],
                                                in0=e16[:, 0:ng * 4], scalar1=NEG)
                    l16 = gpo.tile([P, GRP * 4], f32, tag="l16")
                    nc.vector.tensor_tensor(out=l16[:, 0:ng * 4],
                                            in0=e16[:, 0:ng * 4],
                                            in1=t16[:, 0:ng * 4], op=AL.max)
                    m4 = gpo.tile([P, GRP * 132], bf16, tag="m4")
                    nc.scalar.activation(
                        out=m4[:, 0:ng * 132]
                            .rearrange("p (g r) -> p g r", g=ng)[:, :, 128:132],
                        in_=l16[:, 0:ng * 4].rearrange("p (g h) -> p g h", g=ng),
                        func=AF.Exp)
                    g4b = g4[:, 0:ng * rowlen].bitcast(bf16)   # [P, ng*rowlen*2]
                    for j in range(ng):
                        nc.vector.tensor_tensor(
                            out=m4[:, j * 132:j * 132 + 128]
                                .rearrange("p (h c) -> p h c", h=nh),
                            in0=g4b[:, j * rowlen * 2:j * rowlen * 2 + 128]
                                .rearrange("p (h c) -> p h c", h=nh),
                            in1=m4[:, j * 132 + 128:j * 132 + 128 + nh]
                                .rearrange("p (h o) -> p h o", o=1)
                                .to_broadcast([P, nh, 128 // nh]),
                            op=AL.mult)
                    for j in range(ng):
                        gt = g0 + j
                        b = int(blk_of_tile[gt])
                        first = gt == tsl[b]
                        last = gt == tsl[b + 1] - 1
                        if first:
                            pb = psF.tile([P, mcols], f32, tag="pb")
                        nc.tensor.matmul(out=pb[:],
                                         lhsT=oh4[:, j * P:(j + 1) * P],
                                         rhs=m4[:, j * 132:j * 132 + mcols],
                                         start=first, stop=last)
                        if last:
                            finalize(layer, b, pb)

            def finalize(layer, b, pb):
                rows = min(P, NPC - b * P)
                nh = HEADS if layer == 1 else 1
                den = gpo.tile([P, 4], f32, tag="den")
                nc.vector.tensor_scalar_add(out=den[:, 0:nh],
                                            in0=pb[:, 128:128 + nh],
                                            scalar1=1e-16)
                rec = gpo.tile([P, 4], f32, tag="rec")
                nc.vector.reciprocal(out=rec[:, 0:nh], in_=den[:, 0:nh])
                if layer == 1:
                    tmp = sbp.tile([P, 128], f32, tag="tmp")
                    nc.vector.tensor_tensor(
                        out=tmp[:].rearrange("p (h c) -> p h c", h=nh),
                        in0=pb[:, 0:128].rearrange("p (h c) -> p h c", h=nh),
                        in1=rec[:, 0:nh].rearrange("p (h o) -> p h o", o=1)
                            .to_broadcast([P, nh, 128 // nh]),
                        op=AL.mult)
                    nc.vector.tensor_tensor(out=tmp[:], in0=tmp[:], in1=b1bc[:],
                                            op=AL.add)
                    nc.vector.tensor_scalar_max(out=tmp[:], in0=tmp[:], scalar1=0.0)
                    upk = sbp.tile([P, ROW1], f32, tag="upk")  # ROW1>=ROW2
                    nc.vector.tensor_copy(out=upk[:, 0:64].bitcast(bf16), in_=tmp[:])
                    pt = psF.tile([P, P], f32, tag="pt")
                    nc.tensor.transpose(out=pt[:], in_=tmp[:], identity=ident[:])
                    uT = sbp.tile([P, P], bf16, tag="uT")
                    nc.vector.tensor_copy(out=uT[:], in_=pt[:])
                    pa2 = psE.tile([P, 2], f32, tag="pD")
                    nc.tensor.matmul(out=pa2[:], lhsT=uT[:], rhs=vsd[:],
                                     start=True, stop=True)
                    nc.vector.tensor_copy(out=upk[:, 64:65], in_=pa2[:, 0:1])
                    nc.vector.tensor_copy(out=adst2[:, b:b + 1], in_=pa2[:, 1:2])
                    nc.sync.dma_start(out=ltab2[b * P:b * P + rows, :],
                                      in_=upk[0:rows, 0:ROW2])
                    if b % 7 == 6:
                        k = b // 7
                        nc.gpsimd.collective_compute(
                            "AllGather", mybir.AluOpType.bypass,
                            replica_groups=[list(range(C))],
                            ins=[ltab2[k * 896:(k + 1) * 896, :]],
                            outs=[tab2[k * 7168:(k + 1) * 7168, :]])
                else:
                    agg = sbp.tile([P, P], f32, tag="tmp")
                    nc.vector.tensor_scalar_mul(out=agg[:], in0=pb[:, 0:128],
                                                scalar1=rec[:, 0:1])
                    pt = psF.tile([P, P], f32, tag="pt")
                    nc.tensor.transpose(out=pt[:], in_=agg[:], identity=ident[:])
                    aT = sbp.tile([P, P], bf16, tag="uT")
                    nc.vector.tensor_copy(out=aT[:], in_=pt[:])
                    pz = psE.tile([P, 16], f32, tag="pD")
                    nc.tensor.matmul(out=pz[:], lhsT=aT[:], rhs=W2_sb,
                                     start=True, stop=True)
                    z = gpo.tile([P, 16], f32, tag="z")
                    nc.vector.tensor_tensor(out=z[:], in0=pz[:], in1=b2bc[:],
                                            op=AL.add)
                    mx = gpo.tile([P, 1], f32, tag="mx")
                    nc.vector.tensor_reduce(out=mx[:], in_=z[:],
                                            axis=mybir.AxisListType.X,
                                            op=AL.max, negate=True)  # -max
                    es = gpo.tile([P, 16], f32, tag="es")
                    ssum = gpo.tile([P, 1], f32, tag="ssum")
                    nc.scalar.activation(out=es[:], in_=z[:], func=AF.Exp,
                                         bias=mx[:], accum_out=ssum[:])
                    ls = gpo.tile([P, 1], f32, tag="ls")
                    nc.scalar.activation(out=ls[:], in_=ssum[:], func=AF.Ln)
                    sh = gpo.tile([P, 1], f32, tag="sh")
                    nc.vector.tensor_tensor(out=sh[:], in0=ls[:], in1=mx[:],
                                            op=AL.subtract)  # ln(s) - (-max)... see note
                    res = gpo.tile([P, 16], f32, tag="res")
                    nc.vector.tensor_scalar_sub(out=res[:], in0=z[:],
                                                scalar1=sh[:, 0:1])
                    nc.sync.dma_start(out=out_d[b * P:b * P + rows, :],
                                      in_=res[0:rows, :])

            with nc.named_scope("edge1"):
                edge_phase(1)

            with nc.named_scope("edge2"):
                edge_phase(2)

    _split_excess_waits(nc)
    return nc


# log_softmax shift note: out = z - max - ln(sum(exp(z - max))).
# mx holds -max (negate=True). es = exp(z + mx), ssum = sum(es), ls = ln(ssum).
# shift = max + ls = ls - mx. res = z - shift.


def _host_arrays(x, W1, att_src1, att_dst1, b1, W2, att_src2, att_dst2, b2,
                 srcT1, srcT2, dstloc, TT):
    TTg = ((TT + CHT - 1) // CHT) * CHT
    xT = np.zeros((P, XPAD), bfnp)
    xT[:, 0:N] = np.asarray(x, np.float32).T.astype(bfnp)

    x_own = np.zeros((C, P, PADN), bfnp)
    for c in range(C):
        end = min(c * NPC + PADN, XPAD)
        x_own[c, :, 0:end - c * NPC] = xT[:, c * NPC:end]

    iota_row = np.broadcast_to(np.arange(P, dtype=np.int32)[None, :], (P, P))
    iota_col = np.arange(P, dtype=np.int32).reshape(P, 1)
    idxs = np.zeros((C, P, 3 * TT + P + 1), np.int32)
    for c in range(C):
        idxs[c, :, 0:TT] = srcT1[c].T
        idxs[c, :, TT:2 * TT] = srcT2[c].T
        idxs[c, :, 2 * TT:3 * TT] = dstloc[c].T
        idxs[c, :, 3 * TT:3 * TT + P] = iota_row
        idxs[c, :, 3 * TT + P:] = iota_col

    dstrow = np.zeros((C, 1, TTg * P), bfnp)
    for c in range(C):
        dstrow[c, 0, 0:TT * P] = dstloc[c].reshape(-1).astype(bfnp)

    NCBF = 128 + 128 + 8 + 16 + 128 + 2 + 128 + TT + 128
    cbf = np.zeros((C, P, NCBF), bfnp)
    W1f = np.asarray(W1, np.float32)
    cbf[:, :, 0:128] = W1f.astype(bfnp)
    cbf[:, :, 128:256] = W1f.T.astype(bfnp)
    ablk = np.zeros((128, 8), np.float32)
    for h in range(HEADS):
        ablk[h * HID:(h + 1) * HID, h] = np.asarray(att_src1, np.float32)[h]
        ablk[h * HID:(h + 1) * HID, 4 + h] = np.asarray(att_dst1, np.float32)[h]
    cbf[:, :, 256:264] = ablk.astype(bfnp)
    W2f = np.asarray(W2, np.float32)
    cbf[:, :, 264:280] = W2f.astype(bfnp)
    cbf[:, 0:16, 280:408] = W2f.T.astype(bfnp)
    cbf[:, 0:16, 408:409] = np.asarray(att_src2, np.float32).T.astype(bfnp)
    cbf[:, 0:16, 409:410] = np.asarray(att_dst2, np.float32).T.astype(bfnp)
    cbf[:, 0:1, 410:538] = np.ones((1, 128), bfnp)
    for c in range(C):
        cbf[c, :, 538:538 + TT] = np.ascontiguousarray(dstloc[c].T).astype(bfnp)
    cbf[:, :, 538 + TT:538 + TT + 128] = np.broadcast_to(
        np.arange(128, dtype=np.float32)[None, None, :], (C, P, 128)).astype(bfnp)

    NCF = 128 + 16 + ROW1 + ROW2 + 128
    cf = np.zeros((1, NCF), np.float32)
    cf[0, 0:128] = np.asarray(b1, np.float32)
    cf[0, 128:144] = np.asarray(b2, np.float32)
    pad1 = np.zeros(ROW1, np.float32)
    pad1[64:68] = -1e9
    cf[0, 144:144 + ROW1] = pad1
    pad2 = np.zeros(ROW2, np.float32)
    pad2[64] = -1e9
    cf[0, 144 + ROW1:144 + ROW1 + ROW2] = pad2
    cf[0, 144 + ROW1 + ROW2:NCF] = 1.0

    return xT, x_own, idxs, dstrow, cbf, cf


_CACHE = {}


def kernel(x, edge_index, W1, att_src1, att_dst1, b1, W2, att_src2, att_dst2, b2,
           _trace=False, _tmpdir=None):
    srcT1, srcT2, dstloc, TB, TT, tile_starts = _prep(np.asarray(edge_index))
    xT, x_own, idxs, dstrow, cbf, cf = _host_arrays(
        x, W1, att_src1, att_dst1, b1, W2, att_src2, att_dst2, b2,
        srcT1, srcT2, dstloc, TT)

    key = (TT, tuple(int(t) for t in TB))
    if key not in _CACHE:
        _CACHE[key] = _build(TT, TB, tile_starts)
    nc = _CACHE[key]

    in_maps = []
    for c in range(C):
        in_maps.append({
            "x_own": np.ascontiguousarray(x_own[c]),
            "idxs": np.ascontiguousarray(idxs[c]),
            "dstrow": np.ascontiguousarray(dstrow[c]),
            "cbf": np.ascontiguousarray(cbf[c]), "cf": cf,
        })

    res = run_bass_kernel_spmd(nc, in_maps, list(range(C)), trace=_trace,
                               tmpdir=_tmpdir)
    out = np.concatenate([res.results[c]["out"] for c in range(C)], axis=0)
    kernel.last_results = res
    return out.astype(np.float32)
